# revision 17
# baseline (speedup 1.0000x reference)
"""BachNet beam-search inference kernel for 8 TRN2 NeuronCores.

Strategy (single NEFF launch, tensor-parallel over the hidden dim):
  - N == P == 62, so stage-1's sort only reorders rows; stages are computed in
    natural pitch order and the one-hot concatenations become row-slices /
    row-gathers of the first-layer weight matrices.
  - Each core owns a 256-wide column shard of every w1/w2; w3 is replicated.
    The x @ w1 mat-vecs run on VectorE as fused multiply-reduce over
    transposed weight images; the batched layer-2 GEMMs run on TensorE.
    One AllGather shares layer-1 activations, a second shares layer-2
    activations (logits are then computed locally from replicated w3).
  - The stage-2 top-62 selection runs fully on-device and replicated: a
    3-round 62-ary probe search (ScalarE sign-count against a broadcast
    copy of the flattened scores) finds a threshold with exactly 62
    elements above it; triangular matmuls turn the mask into row-major
    compaction ranks, and a gpsimd local_scatter builds the alto one-hot.
  - The final (stage-3) top-62 + sort runs on host from the tiny [62,62]
    result matrices (exact, matches jnp.argsort tie-breaking).
  - selu is computed as lam*relu(v) + lam*alpha*(exp(min(v,0))-1) with the
    lam factor pre-folded into the layer-1/2 weights on host.
"""
import sys

sys.path.insert(0, "/opt/trn_rl_repo")

import numpy as np
import ml_dtypes

import concourse.bacc as bacc
import concourse.tile as tile
import concourse.mybir as mybir
from concourse import bass_utils

P = 62           # pitch classes == num candidates
D = 10112        # bass input dim (= 79 * 128)
H = 2048         # hidden
NCORES = 8
HS = H // NCORES          # 256 hidden columns per core
KT2 = H // 128            # 16 k-tiles for layer 2
MT = HS // 128            # 2 m-tiles per core shard
CHK = 2528                # layer-1 k-chunk (D = 4*2528)
NCH = D // CHK            # 4 chunks per h-tile
LAM = 1.0507009873554805
ALPHA = 1.6732632423543772
LA = LAM * ALPHA
FLAT = P * P              # 3844

f32 = mybir.dt.float32
bf16 = mybir.dt.bfloat16
i16 = mybir.dt.int16
OP = mybir.AluOpType
AX = mybir.AxisListType
AF = mybir.ActivationFunctionType
RG = [list(range(NCORES))]


def _build():
    nc = bacc.Bacc("TRN2", target_bir_lowering=False, debug=False,
                   num_devices=NCORES)

    def din(name, shape, dtype=f32):
        return nc.dram_tensor(name, shape, dtype, kind="ExternalInput")

    xb_d = din("xbi", [128, D])
    w1_d = {s: din(f"{s}w1t", [128, MT * D]) for s in "bat"}
    w2_d = {s: din(f"{s}w2i", [128, KT2 * HS]) for s in "bat"}
    w3_d = {s: din(f"{s}w3i", [128, KT2 * P]) for s in "bat"}
    aohT_d = din("aohT", [128, MT * P])
    tohb_d = din("tohb", [P, HS])
    toha_d = din("toha", [P, HS])
    b1_d = {s: din(f"{s}b1c", [128, MT]) for s in "bat"}
    b2_d = {s: din(f"{s}b2r", [1, HS]) for s in "bat"}
    b3_d = {s: din(f"{s}b3r", [1, P]) for s in "bat"}
    ident_d = din("ident", [128, 128])
    LT_d = din("LTc", [P, P])
    SLT_d = din("SLTc", [P, P])
    iotaF_d = din("iotaF", [P, P])
    iotaC_d = din("iotaC", [P, 1])
    iotaC1_d = din("iotaC1", [P, 1])
    onesR_d = din("onesR", [1, HS])
    onesCbf_d = din("onesCbf", [P, 1], bf16)
    iotaFbf_d = din("iotaFbf", [64, P], bf16)

    pa_out = nc.dram_tensor("pa_out", [P, P], f32, kind="ExternalOutput")
    pt_out = nc.dram_tensor("pt_out", [P, P], f32, kind="ExternalOutput")

    with tile.TileContext(nc) as tc:
        with (
            tc.tile_pool(name="consts", bufs=1) as cp,
            tc.tile_pool(name="stream", bufs=2) as sp,
            tc.tile_pool(name="mvscr", bufs=2) as scrp,
            tc.tile_pool(name="work", bufs=1) as wp,
            tc.tile_pool(name="trans", bufs=3) as tp,
            tc.tile_pool(name="ptp", bufs=2, space="PSUM") as pp_tp,
            tc.tile_pool(name="pl1", bufs=2, space="PSUM") as pp_l1,
            tc.tile_pool(name="psel", bufs=2, space="PSUM") as pp_sel,
            tc.tile_pool(name="dram", bufs=1, space="DRAM") as dp,
        ):
            def cload(src, shape, dtype=f32, eng=None):
                t = cp.tile(shape, dtype, tag=src.name, name="c_" + src.name)
                (eng or nc.sync).dma_start(t[:], src[:])
                return t

            # --- small constants (sync queue, ahead of the weight stream) ---
            idn = cload(ident_d, [128, 128])
            lt = cload(LT_d, [P, P])
            slt = cload(SLT_d, [P, P])
            iof = cload(iotaF_d, [P, P])
            ioc = cload(iotaC_d, [P, 1])
            ioc1 = cload(iotaC1_d, [P, 1])
            onr = cload(onesR_d, [1, HS])
            ocb = cload(onesCbf_d, [P, 1], bf16)
            iofb = cload(iotaFbf_d, [64, P], bf16)
            b1s = {s: cload(b1_d[s], [128, MT]) for s in "bat"}
            b2s = {s: cload(b2_d[s], [1, HS]) for s in "bat"}
            b3s = {s: cload(b3_d[s], [1, P]) for s in "bat"}

            # --- warmup collective: trigger ASAP on uninitialized dram ---
            wbi = dp.tile([16, 32], f32, tag="wbi")
            wbo = dp.tile([128, 32], f32, tag="wbo")
            nc.gpsimd.collective_compute(
                "AllGather", OP.bypass, replica_groups=RG,
                ins=[wbi[:].opt()], outs=[wbo[:].opt()])
            # readback on ScalarE's queue so it doesn't head-of-line block
            # the gpsimd DMA stream while the collective stack initializes

            # --- x broadcast [128, D]: host-prepared, contiguous DMA on the
            # fast weight-stream queue, sliced so chunk ci only waits on its
            # own quarter ---
            xb = wp.tile([128, D], f32, tag="xb")
            for ci in range(NCH):
                nc.sync.dma_start(xb[:, ci * CHK:(ci + 1) * CHK],
                                  xb_d[:, ci * CHK:(ci + 1) * CHK])

            aohT = cload(aohT_d, [128, MT * P])
            tohb = cload(tohb_d, [P, HS])
            toha = cload(toha_d, [P, HS])


            # --- layer-1 mat-vec: fused mul+reduce over [128, D] rows ---
            # sh[h] = lam * (x @ w1[:, col_h] + b1[col_h]); w1t image rows = h
            def matvec(s):
                # per chunk: VectorE elementwise product, ScalarE free-axis
                # accumulate (Identity activation with accum_out); the two
                # engines pipeline chunk-to-chunk under the DMA stream.
                cols = []
                for mt in range(MT):
                    accs = [wp.tile([128, 1], f32, tag=f"ac_{s}{mt}{i}",
                                    name=f"ac_{s}{mt}{i}")
                            for i in range(NCH)]
                    for ci in range(NCH):
                        ck = sp.tile([128, CHK], f32, tag="w1ck", name="w1ck")
                        nc.sync.dma_start(
                            ck[:],
                            w1_d[s][:, mt * D + ci * CHK:mt * D + (ci + 1) * CHK])
                        prod = scrp.tile([128, CHK], f32, tag="mvscr",
                                         name="mvscr")
                        nc.vector.tensor_mul(prod[:], ck[:],
                                             xb[:, ci * CHK:(ci + 1) * CHK])
                        nc.scalar.activation(prod[:], prod[:], AF.Identity,
                                             accum_out=accs[ci][:])
                    p01 = tp.tile([128, 1], f32, tag="mvp0", name="p01")
                    nc.vector.tensor_add(p01[:], accs[0][:], accs[1][:])
                    p23 = tp.tile([128, 1], f32, tag="mvp1", name="p23")
                    nc.vector.tensor_add(p23[:], accs[2][:], accs[3][:])
                    p03 = tp.tile([128, 1], f32, tag="mvp2", name="p03")
                    nc.vector.tensor_add(p03[:], p01[:], p23[:])
                    scol = wp.tile([128, 1], f32, tag=f"shc_{s}{mt}",
                                   name=f"shc_{s}{mt}")
                    nc.vector.tensor_add(scol[:], p03[:],
                                         b1s[s][:, mt:mt + 1])
                    cols.append(scol)
                return cols

            # selu: dst = lam*relu(pre) + lam*alpha*(exp(min(pre,0))-1)
            def selu_chain(pre_ap, shcol, parts, width, tag):
                shp = [parts, width]
                m = tp.tile(shp, f32, tag="selu_m", name="selu_m")
                r = tp.tile(shp, f32, tag="selu_r", name="selu_r")
                e = tp.tile(shp, f32, tag="selu_e", name="selu_e")
                e2 = tp.tile(shp, f32, tag="selu_e2", name="selu_e2")
                dst = wp.tile(shp, f32, tag=tag, name=tag)
                if shcol is None:
                    nc.vector.tensor_scalar(m[:], pre_ap, 0.0, None, OP.min)
                    nc.vector.tensor_scalar(r[:], pre_ap, 0.0, None, OP.max)
                else:
                    nc.vector.tensor_scalar(m[:], pre_ap, shcol, 0.0, OP.add,
                                            OP.min)
                    nc.vector.tensor_scalar(r[:], pre_ap, shcol, 0.0, OP.add,
                                            OP.max)
                nc.scalar.activation(e[:], m[:], AF.Exp, scale=1.0 / LAM)
                nc.vector.tensor_scalar(e2[:], e[:], LA, -LA, OP.mult, OP.add)
                nc.vector.tensor_add(dst[:], r[:], e2[:])
                return dst

            # ---------------- stage 1+2 layer 1 (bass || alto) ----------
            shb = matvec("b")
            sha = matvec("a")
            w2s = {s: cload(w2_d[s], [128, KT2 * HS]) for s in "ba"}
            w3s = {s: cload(w3_d[s], [128, KT2 * P]) for s in "ba"}
            h1b = [selu_chain(shb[mt][:], None, 128, 1, f"h1b{mt}")
                   for mt in range(MT)]
            h1a = [selu_chain(aohT[:, mt * P:(mt + 1) * P], sha[mt][:], 128, P,
                              f"h1a{mt}")
                   for mt in range(MT)]

            W1 = P + 1
            hb1 = dp.tile([HS, W1], f32, tag="hb1")
            for mt in range(MT):
                nc.gpsimd.dma_start(hb1[mt * 128:(mt + 1) * 128, 0:P],
                                    h1a[mt][:])
                nc.gpsimd.dma_start(hb1[mt * 128:(mt + 1) * 128, P:W1],
                                    h1b[mt][:])
            ghb1 = dp.tile([H, W1], f32, tag="ghb1")
            nc.gpsimd.collective_compute(
                "AllGather", OP.bypass, replica_groups=RG,
                ins=[hb1[:].opt()], outs=[ghb1[:].opt()])
            H1T = wp.tile([128, KT2 * W1], f32, tag="HT", bufs=2, name="H1T")
            nc.gpsimd.dma_start(
                H1T[:].rearrange("p (kt w) -> p kt w", w=W1),
                ghb1[:].rearrange("(kt p) w -> p kt w", p=128))
            wg = wp.tile([128, 32], f32, tag="warm2")
            nc.gpsimd.dma_start(wg[:], wbo[:])

            # --- TensorE HAM warmup: ~26us of dummy matmuls that start
            # once h1a is ready (i.e. during AllGather-1), so layer 2 runs
            # at the full 2.4 GHz clock ---
            pwarm = pp_l1.tile([P, 512], f32, tag="warmmm", name="pwarm")
            for wi in range(30):
                nc.tensor.matmul(pwarm[:], h1a[0][:], xb[:, :512],
                                 start=(wi == 0), stop=(wi == 29))
            wanc2 = wp.tile([P, 1], f32, tag="wanc2")
            nc.vector.tensor_scalar(wanc2[:], pwarm[:P, 0:1], 1e38, None,
                                    OP.is_ge)

            # ------------- stage 3 layer-1 mat-vec (independent) ---------
            sht = matvec("t")
            w2s["t"] = cp.tile([128, KT2 * HS], f32, tag="tw2i", name="c_tw2i")
            nc.sync.dma_start(w2s["t"][:], w2_d["t"][:])
            w3s["t"] = cp.tile([128, KT2 * P], f32, tag="tw3i", name="c_tw3i")
            nc.sync.dma_start(w3s["t"][:], w3_d["t"][:])

            # ---------------- stage 1+2 layer 2 + logits ----------------
            h2a = []
            h2b = []
            for mt in range(MT):
                pya = pp_l1.tile([128, P], f32, tag="l2", name="pya")
                for kt in range(KT2):
                    nc.tensor.matmul(
                        pya[:],
                        w2s["a"][:, kt * HS + mt * 128:kt * HS + (mt + 1) * 128],
                        H1T[:, kt * W1:kt * W1 + P],
                        start=(kt == 0), stop=False)
                nc.tensor.matmul(pya[:], b2s["a"][:1, mt * 128:(mt + 1) * 128],
                                 onr[:1, :P], start=False, stop=True)
                h2a.append(selu_chain(pya[:], None, 128, P, f"h2a{mt}"))
                pyb = pp_tp.tile([128, 1], f32, tag="tp", name="pyb")
                for kt in range(KT2):
                    nc.tensor.matmul(
                        pyb[:],
                        w2s["b"][:, kt * HS + mt * 128:kt * HS + (mt + 1) * 128],
                        H1T[:, kt * W1 + P:kt * W1 + W1],
                        start=(kt == 0), stop=False)
                nc.tensor.matmul(pyb[:], b2s["b"][:1, mt * 128:(mt + 1) * 128],
                                 onr[:1, :1], start=False, stop=True)
                h2b.append(selu_chain(pyb[:], None, 128, 1, f"h2b{mt}"))

            # AllGather h2 (fused alto+bass), then local logits vs full w3
            hb2 = dp.tile([HS, W1], f32, tag="hb2")
            for mt in range(MT):
                nc.gpsimd.dma_start(hb2[mt * 128:(mt + 1) * 128, 0:P],
                                    h2a[mt][:])
                nc.gpsimd.dma_start(hb2[mt * 128:(mt + 1) * 128, P:W1],
                                    h2b[mt][:])
            ghb2 = dp.tile([H, W1], f32, tag="ghb2")
            nc.gpsimd.collective_compute(
                "AllGather", OP.bypass, replica_groups=RG,
                ins=[hb2[:].opt()], outs=[ghb2[:].opt()])
            H2T = wp.tile([128, KT2 * W1], f32, tag="HT", bufs=2, name="H2T")
            nc.sync.dma_start(
                H2T[:].rearrange("p (kt w) -> p kt w", w=W1),
                ghb2[:].rearrange("(kt p) w -> p kt w", p=128))

            plg_a = pp_sel.tile([P, P], f32, tag="ps", name="plg_a")
            for kt in range(KT2):
                nc.tensor.matmul(plg_a[:], H2T[:, kt * W1:kt * W1 + P],
                                 w3s["a"][:, kt * P:(kt + 1) * P],
                                 start=(kt == 0), stop=False)
            nc.tensor.matmul(plg_a[:], onr[:1, :P], b3s["a"][:1, :],
                             start=False, stop=True)
            plg_b = pp_tp.tile([1, P], f32, tag="tp", name="plg_b")
            for kt in range(KT2):
                nc.tensor.matmul(plg_b[:], H2T[:, kt * W1 + P:kt * W1 + W1],
                                 w3s["b"][:, kt * P:(kt + 1) * P],
                                 start=(kt == 0), stop=False)
            nc.tensor.matmul(plg_b[:], onr[:1, :1], b3s["b"][:1, :],
                             start=False, stop=True)

            # fused softmax: alto rows 0..61 at base 0, bass row copied to 64
            NR = 65
            lgcat = wp.tile([NR, P], f32, tag="lgcat")
            nc.vector.memset(lgcat[:], 0.0)
            nc.vector.tensor_copy(lgcat[:P, :], plg_a[:])
            nc.vector.tensor_copy(lgcat[64:NR, :], plg_b[:])
            nm = wp.tile([NR, 1], f32, tag="nm")
            nc.vector.tensor_reduce(nm[:], lgcat[:], axis=AX.X, op=OP.max,
                                    negate=True)
            E = wp.tile([NR, P], f32, tag="E")
            ssum = wp.tile([NR, 1], f32, tag="ssum")
            nc.scalar.activation(E[:], lgcat[:], AF.Exp, bias=nm[:],
                                 accum_out=ssum[:])
            rec = wp.tile([NR, 1], f32, tag="rec")
            nc.vector.reciprocal(rec[:], ssum[:])
            erow = wp.tile([1, P], f32, tag="erow")
            nc.vector.tensor_copy(erow[:], E[64:NR, :])
            rc62 = wp.tile([1, 1], f32, tag="rc62")
            nc.vector.tensor_copy(rc62[:], rec[64:NR, 0:1])
            ptp2 = pp_tp.tile([P, 1], f32, tag="tp", name="ptp2")
            nc.tensor.transpose(ptp2[:], erow[:1, :], idn[:1, :1])
            pbc = pp_tp.tile([P, 1], f32, tag="tp", name="pbc")
            nc.tensor.matmul(pbc[:], onr[:1, :P], rc62[:1, :1],
                             start=True, stop=True)
            v1 = wp.tile([P, 1], f32, tag="v1")
            nc.vector.tensor_mul(v1[:], ptp2[:], rec[:P, :])
            v = wp.tile([P, 1], f32, tag="v")
            nc.vector.tensor_mul(v[:], v1[:], pbc[:])
            # anchor the warmup collective so it isn't dead code
            # (wbo is uninitialized garbage: is_ge maps any bits, incl. NaN,
            # to 0/1, and the huge threshold makes the result 0)
            wanc = wp.tile([P, 1], f32, tag="wanc")
            nc.vector.tensor_scalar(wanc[:], wg[:P, 0:1], 1e38, None, OP.is_ge)
            nc.vector.scalar_tensor_tensor(v[:], wanc[:], 0.0, v[:],
                                           OP.mult, OP.add)
            nc.vector.scalar_tensor_tensor(v[:], wanc2[:], 0.0, v[:],
                                           OP.mult, OP.add)
            PA = wp.tile([P, P], f32, tag="PA")
            nc.vector.tensor_scalar(PA[:], E[:P, :], v[:], None, OP.mult)
            nc.scalar.dma_start(pa_out[:], PA[:])

            # ---------------- on-device top-62 selection ----------------
            # (1) broadcast flat scores to all partitions: R[i, e] = PA_flat[e]
            paf = dp.tile([P, P], f32, tag="paf")
            nc.gpsimd.dma_start(paf[:], PA[:])
            R = wp.tile([P, FLAT], f32, tag="R")
            nc.gpsimd.dma_start(
                R[:],
                paf[:].rearrange("a b -> (a b)")[None, :].broadcast_to(
                    [P, FLAT]))
            # (2) initial bracket: lo = 0, hi = max * 1.00001
            rmx = wp.tile([P, 1], f32, tag="rmx")
            nc.vector.tensor_reduce(rmx[:], PA[:], axis=AX.X, op=OP.max)
            prx = pp_tp.tile([1, P], f32, tag="tp", name="prx")
            nc.tensor.transpose(prx[:], rmx[:], idn[:P, :P])
            rxr = wp.tile([1, P], f32, tag="rxr")
            nc.vector.tensor_copy(rxr[:], prx[:])
            vmx = wp.tile([1, 1], f32, tag="vmx")
            nc.vector.tensor_reduce(vmx[:], rxr[:], axis=AX.X, op=OP.max)
            nc.vector.tensor_scalar(vmx[:], vmx[:], 1.00001, None, OP.mult)
            phi = pp_tp.tile([P, 1], f32, tag="tp", name="phi")
            nc.tensor.matmul(phi[:], onr[:1, :P], vmx[:1, :1], start=True,
                             stop=True)
            hi = wp.tile([P, 1], f32, tag="hi")
            nc.vector.tensor_copy(hi[:], phi[:])
            lo = wp.tile([P, 1], f32, tag="lo")
            nc.vector.memset(lo[:], 0.0)
            tstar = wp.tile([P, 1], f32, tag="tstar")
            nc.vector.memset(tstar[:], 0.0)
            sgn = wp.tile([P, FLAT], f32, tag="sgn")
            BIG = 1.0e30

            def preduce(vec_ap, op, name):
                # [P,1] -> scalar [1,1] via transpose + free reduce
                pt_ = pp_tp.tile([1, P], f32, tag="tp", name=f"pt_{name}")
                nc.tensor.transpose(pt_[:], vec_ap, idn[:P, :P])
                row = tp.tile([1, P], f32, tag="prow", name="prow")
                nc.vector.tensor_copy(row[:], pt_[:])
                sc_ = tp.tile([1, 1], f32, tag="pscl", name="pscl")
                nc.vector.tensor_reduce(sc_[:], row[:], axis=AX.X, op=op)
                return sc_

            def bcast_col(scalar_ap, name):
                pb_ = pp_tp.tile([P, 1], f32, tag="tp", name=f"pb_{name}")
                nc.tensor.matmul(pb_[:], onr[:1, :P], scalar_ap, start=True,
                                 stop=True)
                return pb_

            for rnd in range(2):
                # probes t_i = lo + (i+1)*(hi-lo)/63
                stp = tp.tile([P, 1], f32, tag="stp", name="stp")
                nc.vector.tensor_sub(stp[:], hi[:], lo[:])
                nc.vector.tensor_scalar(stp[:], stp[:], 1.0 / 63.0, None,
                                        OP.mult)
                tcol = tp.tile([P, 1], f32, tag="tcol", name="tcol")
                nc.vector.scalar_tensor_tensor(tcol[:], ioc1[:], stp[:],
                                               lo[:], OP.mult, OP.add)
                nbt = tp.tile([P, 1], f32, tag="nbt", name="nbt")
                nc.vector.tensor_scalar(nbt[:], tcol[:], -1.0, None, OP.mult)
                ssg = tp.tile([P, 1], f32, tag="ssg", name="ssg")
                nc.scalar.activation(sgn[:], R[:], AF.Sign, bias=nbt[:],
                                     accum_out=ssg[:])
                cnt = tp.tile([P, 1], f32, tag="cnt", name="cnt")
                nc.vector.tensor_scalar(cnt[:], ssg[:], 0.5, FLAT / 2.0,
                                        OP.mult, OP.add)
                # candidate columns: [lo-cand, -hi-cand, t*-cand]
                cand = tp.tile([P, 4], f32, tag="cand", name="cand")
                mlo = tp.tile([P, 1], f32, tag="mlo", name="mlo")
                nc.vector.tensor_scalar(mlo[:], cnt[:], 62.75, None, OP.is_ge)
                nc.vector.tensor_mul(cand[:, 0:1], tcol[:], mlo[:])
                mhi = tp.tile([P, 1], f32, tag="mhi", name="mhi")
                nc.vector.tensor_scalar(mhi[:], cnt[:], 62.25, None, OP.is_le)
                hc = tp.tile([P, 1], f32, tag="hc", name="hc")
                nc.vector.tensor_mul(hc[:], tcol[:], mhi[:])
                hc2 = tp.tile([P, 1], f32, tag="hc2", name="hc2")
                nc.vector.tensor_scalar(hc2[:], mhi[:], BIG, -BIG, OP.mult,
                                        OP.add)
                # cand1 = -(t*mhi + BIG*(1-mhi)) = hc2 - hc   (hc2 above is
                # mhi*BIG - BIG = -(BIG*(1-mhi)))
                nc.vector.tensor_sub(cand[:, 1:2], hc2[:], hc[:])
                c62 = tp.tile([P, 1], f32, tag="c62", name="c62")
                nc.vector.tensor_scalar(c62[:], cnt[:], -62.0, None, OP.add)
                sq = tp.tile([P, 1], f32, tag="sq", name="sq")
                nc.vector.tensor_mul(sq[:], c62[:], c62[:])
                meq = tp.tile([P, 1], f32, tag="meq", name="meq")
                nc.vector.tensor_scalar(meq[:], sq[:], 0.07, None, OP.is_le)
                nc.vector.tensor_mul(cand[:, 2:3], tcol[:], meq[:])
                nc.vector.memset(cand[:, 3:4], 0.0)
                # one transpose + one max-reduce handles all three updates
                pcd = pp_tp.tile([4, P], f32, tag="tp", name=f"pcd{rnd}")
                nc.tensor.transpose(pcd[:], cand[:], idn[:P, :P])
                cdr = tp.tile([4, P], f32, tag="cdr", name="cdr")
                nc.vector.tensor_copy(cdr[:], pcd[:])
                mx3 = tp.tile([4, 1], f32, tag="mx3", name="mx3")
                nc.vector.tensor_reduce(mx3[:], cdr[:], axis=AX.X, op=OP.max)
                mx3r = tp.tile([1, 4], f32, tag="mx3r", name="mx3r")
                pmx3 = pp_tp.tile([1, 4], f32, tag="tp", name=f"pmx3{rnd}")
                nc.tensor.transpose(pmx3[:], mx3[:], idn[:4, :4])
                nc.vector.tensor_copy(mx3r[:], pmx3[:])
                pbc3 = pp_tp.tile([P, 4], f32, tag="tp", name=f"pbc3{rnd}")
                nc.tensor.matmul(pbc3[:], onr[:1, :P], mx3r[:1, :],
                                 start=True, stop=True)
                nc.vector.tensor_max(lo[:], lo[:], pbc3[:, 0:1])
                nhi = tp.tile([P, 1], f32, tag="nhi", name="nhi")
                nc.vector.tensor_scalar(nhi[:], pbc3[:, 1:2], -1.0, None,
                                        OP.mult)
                nc.vector.tensor_tensor(hi[:], hi[:], nhi[:], OP.min)
                nc.vector.tensor_max(tstar[:], tstar[:], pbc3[:, 2:3])
            # (3) mask / compaction ranks / one-hots (verified scheme)
            mask = wp.tile([P, P], f32, tag="mask")
            nc.vector.tensor_scalar(mask[:], PA[:], tstar[:], None, OP.is_gt)
            pmT = pp_sel.tile([P, P], f32, tag="ps", name="pmT")
            nc.tensor.transpose(pmT[:], mask[:], idn[:P, :P])
            mT = wp.tile([P, P], f32, tag="mT")
            nc.vector.tensor_copy(mT[:], pmT[:])
            prc = pp_sel.tile([P, P], f32, tag="ps", name="prc")
            nc.tensor.matmul(prc[:], mT[:], lt[:], start=True, stop=True)
            rcm = wp.tile([P, P], f32, tag="rcm")
            nc.vector.tensor_copy(rcm[:], prc[:])
            pro = pp_tp.tile([1, P], f32, tag="tp", name="pro")
            nc.tensor.matmul(pro[:], rcm[:, P - 1:P], slt[:], start=True,
                             stop=True)
            ror = wp.tile([1, P], f32, tag="ror")
            nc.vector.tensor_copy(ror[:], pro[:])
            proc = pp_tp.tile([P, 1], f32, tag="tp", name="proc")
            nc.tensor.transpose(proc[:], ror[:1, :], idn[:1, :1])
            roc = wp.tile([P, 1], f32, tag="roc")
            nc.vector.tensor_copy(roc[:], proc[:])
            re_ = wp.tile([P, 1], f32, tag="re")
            nc.vector.tensor_add(re_[:], roc[:], rcm[:, P - 1:P])
            g1 = tp.tile([P, P], f32, tag="selu_m", name="g1")
            nc.vector.tensor_scalar(g1[:], iof[:], roc[:], None, OP.is_ge)
            g2 = tp.tile([P, P], f32, tag="selu_r", name="g2")
            nc.vector.tensor_scalar(g2[:], iof[:], re_[:], None, OP.is_lt)
            bb = wp.tile([P, P], f32, tag="bb")
            nc.vector.tensor_mul(bb[:], g1[:], g2[:])
            t1 = tp.tile([P, P], f32, tag="selu_e", name="t1")
            nc.vector.tensor_scalar(t1[:], rcm[:], roc[:], None, OP.add)
            t2 = tp.tile([P, P], f32, tag="selu_e2", name="t2")
            nc.vector.tensor_mul(t2[:], t1[:], mask[:])
            t3 = tp.tile([P, P], f32, tag="selu_m", name="t3")
            nc.vector.tensor_scalar(t3[:], t2[:], -1.0, None, OP.add)
            idx = wp.tile([64, P], i16, tag="idx")
            nc.vector.memset(idx[:], -1)
            nc.vector.tensor_copy(idx[:P, :], t3[:])
            scx = wp.tile([64, 64], bf16, tag="scx")
            nc.gpsimd.local_scatter(scx[:], iofb[:], idx[:], channels=64,
                                    num_elems=64, num_idxs=P)
            pas = pp_tp.tile([1, P], f32, tag="tp", name="pas")
            nc.tensor.matmul(pas[:], ocb[:], scx[:P, :P], start=True,
                             stop=True)
            asr = wp.tile([1, P], f32, tag="asr")
            nc.vector.tensor_copy(asr[:], pas[:])
            pab = pp_sel.tile([P, P], f32, tag="ps", name="pab")
            nc.tensor.matmul(pab[:], onr[:1, :P], asr[:1, :], start=True,
                             stop=True)
            ba = wp.tile([P, P], f32, tag="ba")
            nc.vector.tensor_scalar(ba[:], pab[:], ioc[:], None, OP.is_equal)
            pz = pp_sel.tile([P, P], f32, tag="ps", name="pz")
            nc.tensor.matmul(pz[:], bb[:], PA[:], start=True, stop=True)
            pbat = pp_sel.tile([P, P], f32, tag="ps", name="pbat")
            nc.tensor.transpose(pbat[:], ba[:], idn[:P, :P])
            bat = wp.tile([P, P], f32, tag="bat")
            nc.vector.tensor_copy(bat[:], pbat[:])
            pmm = tp.tile([P, P], f32, tag="selu_r", name="pmm")
            nc.vector.tensor_mul(pmm[:], pz[:], bat[:])
            pcol = wp.tile([P, 1], f32, tag="pcol")
            nc.vector.tensor_reduce(pcol[:], pmm[:], axis=AX.X, op=OP.add)

            # ---------------- stage 3 (tenor) ----------------
            h1t = []
            for mt in range(MT):
                pg = pp_l1.tile([128, P], f32, tag="l2", name="pg")
                nc.tensor.matmul(pg[:], tohb[:, mt * 128:(mt + 1) * 128],
                                 bb[:], start=True, stop=False)
                nc.tensor.matmul(pg[:], toha[:, mt * 128:(mt + 1) * 128],
                                 ba[:], start=False, stop=True)
                h1t.append(selu_chain(pg[:], sht[mt][:], 128, P, f"h1t{mt}"))

            ht1 = dp.tile([HS, P], f32, tag="ht1")
            for mt in range(MT):
                nc.gpsimd.dma_start(ht1[mt * 128:(mt + 1) * 128, :],
                                    h1t[mt][:])
            ght = dp.tile([H, P], f32, tag="ght")
            nc.gpsimd.collective_compute(
                "AllGather", OP.bypass, replica_groups=RG,
                ins=[ht1[:].opt()], outs=[ght[:].opt()])
            H1tT = wp.tile([128, KT2 * P], f32, tag="HT", bufs=2, name="H1tT")
            nc.sync.dma_start(
                H1tT[:].rearrange("p (kt w) -> p kt w", w=P),
                ght[:].rearrange("(kt p) w -> p kt w", p=128))

            h2t = []
            for mt in range(MT):
                pyt = pp_l1.tile([128, P], f32, tag="l2", name="pyt")
                for kt in range(KT2):
                    nc.tensor.matmul(
                        pyt[:],
                        w2s["t"][:, kt * HS + mt * 128:kt * HS + (mt + 1) * 128],
                        H1tT[:, kt * P:(kt + 1) * P],
                        start=(kt == 0), stop=False)
                nc.tensor.matmul(pyt[:], b2s["t"][:1, mt * 128:(mt + 1) * 128],
                                 onr[:1, :P], start=False, stop=True)
                h2t.append(selu_chain(pyt[:], None, 128, P, f"h2t{mt}"))

            ht2 = dp.tile([HS, P], f32, tag="ht2")
            for mt in range(MT):
                nc.gpsimd.dma_start(ht2[mt * 128:(mt + 1) * 128, :],
                                    h2t[mt][:])
            ght2 = dp.tile([H, P], f32, tag="ght2")
            nc.gpsimd.collective_compute(
                "AllGather", OP.bypass, replica_groups=RG,
                ins=[ht2[:].opt()], outs=[ght2[:].opt()])
            H2tT = wp.tile([128, KT2 * P], f32, tag="HT", bufs=2, name="H2tT")
            nc.sync.dma_start(
                H2tT[:].rearrange("p (kt w) -> p kt w", w=P),
                ght2[:].rearrange("(kt p) w -> p kt w", p=128))

            plg_t = pp_sel.tile([P, P], f32, tag="ps", name="plg_t")
            for kt in range(KT2):
                nc.tensor.matmul(plg_t[:], H2tT[:, kt * P:(kt + 1) * P],
                                 w3s["t"][:, kt * P:(kt + 1) * P],
                                 start=(kt == 0), stop=False)
            nc.tensor.matmul(plg_t[:], onr[:1, :P], b3s["t"][:1, :],
                             start=False, stop=True)
            S3 = wp.tile([P, P], f32, tag="S3")
            nc.vector.tensor_copy(S3[:], plg_t[:])
            nm3 = wp.tile([P, 1], f32, tag="nm3")
            nc.vector.tensor_reduce(nm3[:], S3[:], axis=AX.X, op=OP.max,
                                    negate=True)
            E3 = wp.tile([P, P], f32, tag="E3")
            ssum3 = wp.tile([P, 1], f32, tag="ssum3")
            nc.scalar.activation(E3[:], S3[:], AF.Exp, bias=nm3[:],
                                 accum_out=ssum3[:])
            rec3 = wp.tile([P, 1], f32, tag="rec3")
            nc.vector.reciprocal(rec3[:], ssum3[:])
            vv = wp.tile([P, 1], f32, tag="vv")
            nc.vector.tensor_mul(vv[:], rec3[:], pcol[:])
            PT = wp.tile([P, P], f32, tag="PT")
            nc.vector.tensor_scalar(PT[:], E3[:], vv[:], None, OP.mult)
            nc.scalar.dma_start(pt_out[:], PT[:])

    nc.compile()
    return nc


_NC_CACHE = None


def _get_nc():
    global _NC_CACHE
    if _NC_CACHE is None:
        _NC_CACHE = _build()
    return _NC_CACHE


def _prep_inputs(inputs):
    lam = np.float32(LAM)
    x = np.asarray(inputs["inputs_bass"], np.float32)

    def w1timg(w):
        # [D, 256] -> transposed image [128, MT*D]: img[p, mt*D+k] = w[k, mt*128+p]
        wt = np.ascontiguousarray(w.T)              # [256, D]
        return np.ascontiguousarray(
            wt.reshape(MT, 128, D).transpose(1, 0, 2).reshape(128, MT * D))

    def w2img(w):
        return np.ascontiguousarray(
            w.reshape(KT2, 128, HS).transpose(1, 0, 2).reshape(128, KT2 * HS))

    def w3img(w):
        # [2048, P] -> [128, KT2*P]
        return np.ascontiguousarray(
            w.reshape(KT2, 128, P).transpose(1, 0, 2).reshape(128, KT2 * P))

    def mtimg(w):
        # [256, P] -> [128, MT*P]
        return np.ascontiguousarray(
            w.reshape(MT, 128, P).transpose(1, 0, 2).reshape(128, MT * P))

    base = {
        "ident": np.eye(128, dtype=np.float32),
        "LTc": (np.arange(P)[:, None] <= np.arange(P)[None, :]).astype(np.float32),
        "SLTc": (np.arange(P)[:, None] < np.arange(P)[None, :]).astype(np.float32),
        "iotaF": np.broadcast_to(np.arange(P, dtype=np.float32), (P, P)).copy(),
        "iotaC": np.arange(P, dtype=np.float32)[:, None].copy(),
        "iotaC1": (np.arange(P, dtype=np.float32)[:, None] + 1.0).copy(),
        "onesR": np.ones((1, HS), np.float32),
        "onesCbf": np.ones((P, 1), ml_dtypes.bfloat16),
        "iotaFbf": np.broadcast_to(
            np.arange(P, dtype=ml_dtypes.bfloat16), (64, P)).copy(),
        "xbi": np.ascontiguousarray(np.broadcast_to(x, (128, D))),
    }
    W = {k: np.asarray(v, np.float32) for k, v in inputs.items()}
    in_maps = []
    for c in range(NCORES):
        cols = slice(HS * c, HS * (c + 1))
        m = dict(base)
        for s in "bat":
            m[f"{s}w1t"] = w1timg(lam * W[f"{s}w1"][:D, cols])
            m[f"{s}w2i"] = w2img(lam * W[f"{s}w2"][:, cols])
            m[f"{s}w3i"] = w3img(W[f"{s}w3"])
            m[f"{s}b1c"] = np.ascontiguousarray(
                (lam * W[f"{s}b1"][cols]).reshape(MT, 128).T)
            m[f"{s}b2r"] = (lam * W[f"{s}b2"][cols])[None, :].copy()
            m[f"{s}b3r"] = W[f"{s}b3"][None, :].copy()
        m["aohT"] = mtimg(np.ascontiguousarray(
            (lam * W["aw1"][D:D + P, cols]).T))
        m["tohb"] = np.ascontiguousarray(lam * W["tw1"][D:D + P, cols])
        m["toha"] = np.ascontiguousarray(lam * W["tw1"][D + P:D + 2 * P, cols])
        in_maps.append(m)
    return in_maps


def _postprocess(pa, pt):
    flat = pa.reshape(-1)
    order = np.argsort(-flat, kind="stable")[:P]
    sel = np.sort(order)                  # device rank order = flat position
    j_sel = sel // P
    a_sel = sel % P
    flat3 = pt.reshape(-1)
    idx3 = np.argsort(-flat3, kind="stable")[:P]
    row = idx3 // P
    out = np.stack([
        flat3[idx3],
        j_sel[row].astype(np.float32),
        a_sel[row].astype(np.float32),
        (idx3 % P).astype(np.float32),
    ], axis=1)
    return out


def run(inputs, trace=False):
    nc = _get_nc()
    in_maps = _prep_inputs(inputs)
    res = bass_utils.run_bass_kernel_spmd(
        nc, in_maps, core_ids=list(range(NCORES)), trace=trace)
    r0 = res.results[0]
    out = _postprocess(r0["pa_out"], r0["pt_out"])
    return out, res.exec_time_ns


def kernel(**inputs) -> np.ndarray:
    out, _ = run(inputs, trace=False)
    return out


# revision 19
# speedup vs baseline: 1.0334x; 1.0334x over previous
"""BachNet beam-search inference kernel for 8 TRN2 NeuronCores.

Strategy (single NEFF launch, tensor-parallel over the hidden dim):
  - N == P == 62, so stage-1's sort only reorders rows; stages are computed in
    natural pitch order and the one-hot concatenations become row-slices /
    row-gathers of the first-layer weight matrices.
  - Each core owns a 256-wide column shard of every w1/w2; w3 is replicated.
    The x @ w1 mat-vecs run on VectorE as fused multiply-reduce over
    transposed weight images; the batched layer-2 GEMMs run on TensorE.
    One AllGather shares layer-1 activations, a second shares layer-2
    activations (logits are then computed locally from replicated w3).
  - The stage-2 top-62 selection runs fully on-device and replicated: a
    3-round 62-ary probe search (ScalarE sign-count against a broadcast
    copy of the flattened scores) finds a threshold with exactly 62
    elements above it; triangular matmuls turn the mask into row-major
    compaction ranks, and a gpsimd local_scatter builds the alto one-hot.
  - The final (stage-3) top-62 + sort runs on host from the tiny [62,62]
    result matrices (exact, matches jnp.argsort tie-breaking).
  - selu is computed as lam*relu(v) + lam*alpha*(exp(min(v,0))-1) with the
    lam factor pre-folded into the layer-1/2 weights on host.
"""
import sys

sys.path.insert(0, "/opt/trn_rl_repo")

import numpy as np
import ml_dtypes

import concourse.bacc as bacc
import concourse.tile as tile
import concourse.mybir as mybir
from concourse import bass_utils

P = 62           # pitch classes == num candidates
D = 10112        # bass input dim (= 79 * 128)
H = 2048         # hidden
NCORES = 8
HS = H // NCORES          # 256 hidden columns per core
KT2 = H // 128            # 16 k-tiles for layer 2
MT = HS // 128            # 2 m-tiles per core shard
CHK = 2528                # layer-1 k-chunk (D = 4*2528)
NCH = D // CHK            # 4 chunks per h-tile
LAM = 1.0507009873554805
ALPHA = 1.6732632423543772
LA = LAM * ALPHA
FLAT = P * P              # 3844

f32 = mybir.dt.float32
bf16 = mybir.dt.bfloat16
i16 = mybir.dt.int16
OP = mybir.AluOpType
AX = mybir.AxisListType
AF = mybir.ActivationFunctionType
RG = [list(range(NCORES))]


def _build():
    nc = bacc.Bacc("TRN2", target_bir_lowering=False, debug=False,
                   num_devices=NCORES)

    def din(name, shape, dtype=f32):
        return nc.dram_tensor(name, shape, dtype, kind="ExternalInput")

    xb_d = din("xbi", [128, D])
    w1_d = {s: din(f"{s}w1t", [128, MT * D]) for s in "bat"}
    w2_d = {s: din(f"{s}w2i", [128, KT2 * HS]) for s in "bat"}
    w3_d = {s: din(f"{s}w3i", [128, KT2 * P]) for s in "bat"}
    aohT_d = din("aohT", [128, MT * P])
    tohb_d = din("tohb", [P, HS])
    toha_d = din("toha", [P, HS])
    b1_d = {s: din(f"{s}b1c", [128, MT]) for s in "bat"}
    b2_d = {s: din(f"{s}b2r", [1, HS]) for s in "bat"}
    b3_d = {s: din(f"{s}b3r", [1, P]) for s in "bat"}
    ident_d = din("ident", [128, 128])
    LT_d = din("LTc", [P, P])
    SLT_d = din("SLTc", [P, P])
    iotaF_d = din("iotaF", [P, P])
    iotaC_d = din("iotaC", [P, 1])
    iotaC1_d = din("iotaC1", [P, 1])
    onesR_d = din("onesR", [1, HS])
    onesCbf_d = din("onesCbf", [P, 1], bf16)
    iotaFbf_d = din("iotaFbf", [64, P], bf16)

    pa_out = nc.dram_tensor("pa_out", [P, P], f32, kind="ExternalOutput")
    pt_out = nc.dram_tensor("pt_out", [P, P], f32, kind="ExternalOutput")

    with tile.TileContext(nc) as tc:
        with (
            tc.tile_pool(name="consts", bufs=1) as cp,
            tc.tile_pool(name="stream", bufs=2) as sp,
            tc.tile_pool(name="mvscr", bufs=2) as scrp,
            tc.tile_pool(name="work", bufs=1) as wp,
            tc.tile_pool(name="trans", bufs=3) as tp,
            tc.tile_pool(name="ptp", bufs=2, space="PSUM") as pp_tp,
            tc.tile_pool(name="pl1", bufs=2, space="PSUM") as pp_l1,
            tc.tile_pool(name="psel", bufs=2, space="PSUM") as pp_sel,
            tc.tile_pool(name="pwarm", bufs=1, space="PSUM") as pp_w,
            tc.tile_pool(name="dram", bufs=1, space="DRAM") as dp,
        ):
            def cload(src, shape, dtype=f32, eng=None):
                t = cp.tile(shape, dtype, tag=src.name, name="c_" + src.name)
                (eng or nc.sync).dma_start(t[:], src[:])
                return t

            # --- small constants (sync queue, ahead of the weight stream) ---
            idn = cload(ident_d, [128, 128])
            lt = cload(LT_d, [P, P])
            slt = cload(SLT_d, [P, P])
            iof = cload(iotaF_d, [P, P])
            ioc = cload(iotaC_d, [P, 1])
            ioc1 = cload(iotaC1_d, [P, 1])
            onr = cload(onesR_d, [1, HS])
            ocb = cload(onesCbf_d, [P, 1], bf16)
            iofb = cload(iotaFbf_d, [64, P], bf16)
            b1s = {s: cload(b1_d[s], [128, MT]) for s in "bat"}
            b2s = {s: cload(b2_d[s], [1, HS]) for s in "bat"}
            b3s = {s: cload(b3_d[s], [1, P]) for s in "bat"}

            # --- warmup collective: trigger ASAP on uninitialized dram ---
            wbi = dp.tile([16, 32], f32, tag="wbi")
            wbo = dp.tile([128, 32], f32, tag="wbo")
            nc.gpsimd.collective_compute(
                "AllGather", OP.bypass, replica_groups=RG,
                ins=[wbi[:].opt()], outs=[wbo[:].opt()])
            # readback on ScalarE's queue so it doesn't head-of-line block
            # the gpsimd DMA stream while the collective stack initializes

            # --- x broadcast [128, D]: host-prepared, contiguous DMA on the
            # fast weight-stream queue, sliced so chunk ci only waits on its
            # own quarter ---
            xb = wp.tile([128, D], f32, tag="xb")
            for ci in range(NCH):
                nc.sync.dma_start(xb[:, ci * CHK:(ci + 1) * CHK],
                                  xb_d[:, ci * CHK:(ci + 1) * CHK])

            aohT = cload(aohT_d, [128, MT * P])
            tohb = cload(tohb_d, [P, HS])
            toha = cload(toha_d, [P, HS])


            # --- layer-1 mat-vec: fused mul+reduce over [128, D] rows ---
            # sh[h] = lam * (x @ w1[:, col_h] + b1[col_h]); w1t image rows = h
            def matvec(s):
                # per chunk: VectorE elementwise product, ScalarE free-axis
                # accumulate (Identity activation with accum_out); the two
                # engines pipeline chunk-to-chunk under the DMA stream.
                cols = []
                for mt in range(MT):
                    accs = [wp.tile([128, 1], f32, tag=f"ac_{s}{mt}{i}",
                                    name=f"ac_{s}{mt}{i}")
                            for i in range(NCH)]
                    for ci in range(NCH):
                        ck = sp.tile([128, CHK], f32, tag="w1ck", name="w1ck")
                        nc.sync.dma_start(
                            ck[:],
                            w1_d[s][:, mt * D + ci * CHK:mt * D + (ci + 1) * CHK])
                        prod = scrp.tile([128, CHK], f32, tag="mvscr",
                                         name="mvscr")
                        nc.vector.tensor_mul(prod[:], ck[:],
                                             xb[:, ci * CHK:(ci + 1) * CHK])
                        nc.scalar.activation(prod[:], prod[:], AF.Identity,
                                             accum_out=accs[ci][:])
                    p01 = tp.tile([128, 1], f32, tag="mvp0", name="p01")
                    nc.vector.tensor_add(p01[:], accs[0][:], accs[1][:])
                    p23 = tp.tile([128, 1], f32, tag="mvp1", name="p23")
                    nc.vector.tensor_add(p23[:], accs[2][:], accs[3][:])
                    p03 = tp.tile([128, 1], f32, tag="mvp2", name="p03")
                    nc.vector.tensor_add(p03[:], p01[:], p23[:])
                    scol = wp.tile([128, 1], f32, tag=f"shc_{s}{mt}",
                                   name=f"shc_{s}{mt}")
                    nc.vector.tensor_add(scol[:], p03[:],
                                         b1s[s][:, mt:mt + 1])
                    cols.append(scol)
                return cols

            # selu: dst = lam*relu(pre) + lam*alpha*(exp(min(pre,0))-1)
            def selu_chain(pre_ap, shcol, parts, width, tag):
                shp = [parts, width]
                m = tp.tile(shp, f32, tag="selu_m", name="selu_m")
                r = tp.tile(shp, f32, tag="selu_r", name="selu_r")
                e = tp.tile(shp, f32, tag="selu_e", name="selu_e")
                e2 = tp.tile(shp, f32, tag="selu_e2", name="selu_e2")
                dst = wp.tile(shp, f32, tag=tag, name=tag)
                if shcol is None:
                    nc.vector.tensor_scalar(m[:], pre_ap, 0.0, None, OP.min)
                    nc.vector.tensor_scalar(r[:], pre_ap, 0.0, None, OP.max)
                else:
                    nc.vector.tensor_scalar(m[:], pre_ap, shcol, 0.0, OP.add,
                                            OP.min)
                    nc.vector.tensor_scalar(r[:], pre_ap, shcol, 0.0, OP.add,
                                            OP.max)
                nc.scalar.activation(e[:], m[:], AF.Exp, scale=1.0 / LAM)
                nc.vector.tensor_scalar(e2[:], e[:], LA, -LA, OP.mult, OP.add)
                nc.vector.tensor_add(dst[:], r[:], e2[:])
                return dst

            # ---------------- stage 1+2 layer 1 (bass || alto) ----------
            W1 = P + 1
            sha = matvec("a")
            w2s = {}
            w3s = {}
            w2s["a"] = cload(w2_d["a"], [128, KT2 * HS])
            w3s["a"] = cload(w3_d["a"], [128, KT2 * P])
            h1a = [selu_chain(aohT[:, mt * P:(mt + 1) * P], sha[mt][:], 128, P,
                              f"h1a{mt}")
                   for mt in range(MT)]
            # AllGather the alto activations alone: they gate the whole chain,
            # while the bass vector joins later (only needed at the PA stage)
            hb1 = dp.tile([HS, P], f32, tag="hb1")
            for mt in range(MT):
                nc.gpsimd.dma_start(hb1[mt * 128:(mt + 1) * 128, :],
                                    h1a[mt][:])
            ghb1 = dp.tile([H, P], f32, tag="ghb1")
            nc.gpsimd.collective_compute(
                "AllGather", OP.bypass, replica_groups=RG,
                ins=[hb1[:].opt()], outs=[ghb1[:].opt()])
            H1aT = wp.tile([128, KT2 * P], f32, tag="HT", bufs=2, name="H1aT")
            for q in range(4):
                nc.gpsimd.dma_start(
                    H1aT[:, q * 4 * P:(q + 1) * 4 * P].rearrange(
                        "p (kt w) -> p kt w", w=P),
                    ghb1[q * 512:(q + 1) * 512, :].rearrange(
                        "(kt p) w -> p kt w", p=128))
            wg = wp.tile([128, 32], f32, tag="warm2")
            nc.gpsimd.dma_start(wg[:], wbo[:])

            shb = matvec("b")
            w2s["b"] = cload(w2_d["b"], [128, KT2 * HS])
            w3s["b"] = cload(w3_d["b"], [128, KT2 * P])
            h1b = [selu_chain(shb[mt][:], None, 128, 1, f"h1b{mt}")
                   for mt in range(MT)]
            hb1b = dp.tile([HS, 1], f32, tag="hb1b")
            for mt in range(MT):
                nc.gpsimd.dma_start(hb1b[mt * 128:(mt + 1) * 128, :],
                                    h1b[mt][:])
            ghb1b = dp.tile([H, 1], f32, tag="ghb1b")
            nc.gpsimd.collective_compute(
                "AllGather", OP.bypass, replica_groups=RG,
                ins=[hb1b[:].opt()], outs=[ghb1b[:].opt()])
            H1bT = wp.tile([128, KT2], f32, tag="H1bT")
            nc.gpsimd.dma_start(
                H1bT[:],
                ghb1b[:].rearrange("(kt p) w -> p (kt w)", p=128))

            # --- TensorE HAM warmup: ~26us of dummy matmuls that start
            # once h1a is ready (i.e. during AllGather-1), so layer 2 runs
            # at the full 2.4 GHz clock ---
            pwarm = pp_w.tile([P, 512], f32, tag="warmmm", name="pwarm")
            for wi in range(30):
                nc.tensor.matmul(pwarm[:], h1a[0][:], xb[:, :512],
                                 start=(wi == 0), stop=(wi == 29))
            wanc2 = wp.tile([P, 1], f32, tag="wanc2")
            nc.vector.tensor_scalar(wanc2[:], pwarm[:P, 0:1], 1e38, None,
                                    OP.is_ge)

            # ------------- stage 3 layer-1 mat-vec (independent) ---------
            sht = matvec("t")
            w2s["t"] = cp.tile([128, KT2 * HS], f32, tag="tw2i", name="c_tw2i")
            nc.sync.dma_start(w2s["t"][:], w2_d["t"][:])
            w3s["t"] = cp.tile([128, KT2 * P], f32, tag="tw3i", name="c_tw3i")
            nc.sync.dma_start(w3s["t"][:], w3_d["t"][:])

            # ---------------- stage 1+2 layer 2 + logits ----------------
            h2a = []
            h2b = []
            for mt in range(MT):
                pya = pp_l1.tile([128, P], f32, tag="l2", name="pya")
                for kt in range(KT2):
                    nc.tensor.matmul(
                        pya[:],
                        w2s["a"][:, kt * HS + mt * 128:kt * HS + (mt + 1) * 128],
                        H1aT[:, kt * P:(kt + 1) * P],
                        start=(kt == 0), stop=False)
                nc.tensor.matmul(pya[:], b2s["a"][:1, mt * 128:(mt + 1) * 128],
                                 onr[:1, :P], start=False, stop=True)
                h2a.append(selu_chain(pya[:], None, 128, P, f"h2a{mt}"))
                pyb = pp_tp.tile([128, 1], f32, tag="tp", name="pyb")
                for kt in range(KT2):
                    nc.tensor.matmul(
                        pyb[:],
                        w2s["b"][:, kt * HS + mt * 128:kt * HS + (mt + 1) * 128],
                        H1bT[:, kt:kt + 1],
                        start=(kt == 0), stop=False)
                nc.tensor.matmul(pyb[:], b2s["b"][:1, mt * 128:(mt + 1) * 128],
                                 onr[:1, :1], start=False, stop=True)
                h2b.append(selu_chain(pyb[:], None, 128, 1, f"h2b{mt}"))

            # AllGather h2 (fused alto+bass), then local logits vs full w3
            hb2 = dp.tile([HS, W1], f32, tag="hb2")
            for mt in range(MT):
                nc.gpsimd.dma_start(hb2[mt * 128:(mt + 1) * 128, 0:P],
                                    h2a[mt][:])
                nc.gpsimd.dma_start(hb2[mt * 128:(mt + 1) * 128, P:W1],
                                    h2b[mt][:])
            ghb2 = dp.tile([H, W1], f32, tag="ghb2")
            nc.gpsimd.collective_compute(
                "AllGather", OP.bypass, replica_groups=RG,
                ins=[hb2[:].opt()], outs=[ghb2[:].opt()])
            H2T = wp.tile([128, KT2 * W1], f32, tag="HT", bufs=2, name="H2T")
            nc.sync.dma_start(
                H2T[:].rearrange("p (kt w) -> p kt w", w=W1),
                ghb2[:].rearrange("(kt p) w -> p kt w", p=128))

            plg_a = pp_sel.tile([P, P], f32, tag="ps", name="plg_a")
            for kt in range(KT2):
                nc.tensor.matmul(plg_a[:], H2T[:, kt * W1:kt * W1 + P],
                                 w3s["a"][:, kt * P:(kt + 1) * P],
                                 start=(kt == 0), stop=False)
            nc.tensor.matmul(plg_a[:], onr[:1, :P], b3s["a"][:1, :],
                             start=False, stop=True)
            plg_b = pp_tp.tile([1, P], f32, tag="tp", name="plg_b")
            for kt in range(KT2):
                nc.tensor.matmul(plg_b[:], H2T[:, kt * W1 + P:kt * W1 + W1],
                                 w3s["b"][:, kt * P:(kt + 1) * P],
                                 start=(kt == 0), stop=False)
            nc.tensor.matmul(plg_b[:], onr[:1, :1], b3s["b"][:1, :],
                             start=False, stop=True)

            # fused softmax: alto rows 0..61 at base 0, bass row copied to 64
            NR = 65
            lgcat = wp.tile([NR, P], f32, tag="lgcat")
            nc.vector.memset(lgcat[:], 0.0)
            nc.vector.tensor_copy(lgcat[:P, :], plg_a[:])
            nc.vector.tensor_copy(lgcat[64:NR, :], plg_b[:])
            nm = wp.tile([NR, 1], f32, tag="nm")
            nc.vector.tensor_reduce(nm[:], lgcat[:], axis=AX.X, op=OP.max,
                                    negate=True)
            E = wp.tile([NR, P], f32, tag="E")
            ssum = wp.tile([NR, 1], f32, tag="ssum")
            nc.scalar.activation(E[:], lgcat[:], AF.Exp, bias=nm[:],
                                 accum_out=ssum[:])
            rec = wp.tile([NR, 1], f32, tag="rec")
            nc.vector.reciprocal(rec[:], ssum[:])
            erow = wp.tile([1, P], f32, tag="erow")
            nc.vector.tensor_copy(erow[:], E[64:NR, :])
            rc62 = wp.tile([1, 1], f32, tag="rc62")
            nc.vector.tensor_copy(rc62[:], rec[64:NR, 0:1])
            ptp2 = pp_tp.tile([P, 1], f32, tag="tp", name="ptp2")
            nc.tensor.transpose(ptp2[:], erow[:1, :], idn[:1, :1])
            pbc = pp_tp.tile([P, 1], f32, tag="tp", name="pbc")
            nc.tensor.matmul(pbc[:], onr[:1, :P], rc62[:1, :1],
                             start=True, stop=True)
            v1 = wp.tile([P, 1], f32, tag="v1")
            nc.vector.tensor_mul(v1[:], ptp2[:], rec[:P, :])
            v = wp.tile([P, 1], f32, tag="v")
            nc.vector.tensor_mul(v[:], v1[:], pbc[:])
            # anchor the warmup collective so it isn't dead code
            # (wbo is uninitialized garbage: is_ge maps any bits, incl. NaN,
            # to 0/1, and the huge threshold makes the result 0)
            wanc = wp.tile([P, 1], f32, tag="wanc")
            nc.vector.tensor_scalar(wanc[:], wg[:P, 0:1], 1e38, None, OP.is_ge)
            nc.vector.scalar_tensor_tensor(v[:], wanc[:], 0.0, v[:],
                                           OP.mult, OP.add)
            nc.vector.scalar_tensor_tensor(v[:], wanc2[:], 0.0, v[:],
                                           OP.mult, OP.add)
            PA = wp.tile([P, P], f32, tag="PA")
            nc.vector.tensor_scalar(PA[:], E[:P, :], v[:], None, OP.mult)
            nc.scalar.dma_start(pa_out[:], PA[:])

            # ---------------- on-device top-62 selection ----------------
            # (1) broadcast flat scores to all partitions: R[i, e] = PA_flat[e]
            paf = dp.tile([P, P], f32, tag="paf")
            nc.gpsimd.dma_start(paf[:], PA[:])
            R = wp.tile([P, FLAT], f32, tag="R")
            nc.gpsimd.dma_start(
                R[:],
                paf[:].rearrange("a b -> (a b)")[None, :].broadcast_to(
                    [P, FLAT]))
            # (2) initial bracket: lo = 0, hi = max * 1.00001
            rmx = wp.tile([P, 1], f32, tag="rmx")
            nc.vector.tensor_reduce(rmx[:], PA[:], axis=AX.X, op=OP.max)
            prx = pp_tp.tile([1, P], f32, tag="tp", name="prx")
            nc.tensor.transpose(prx[:], rmx[:], idn[:P, :P])
            rxr = wp.tile([1, P], f32, tag="rxr")
            nc.vector.tensor_copy(rxr[:], prx[:])
            vmx = wp.tile([1, 1], f32, tag="vmx")
            nc.vector.tensor_reduce(vmx[:], rxr[:], axis=AX.X, op=OP.max)
            nc.vector.tensor_scalar(vmx[:], vmx[:], 1.00001, None, OP.mult)
            phi = pp_tp.tile([P, 1], f32, tag="tp", name="phi")
            nc.tensor.matmul(phi[:], onr[:1, :P], vmx[:1, :1], start=True,
                             stop=True)
            hi = wp.tile([P, 1], f32, tag="hi")
            nc.vector.tensor_copy(hi[:], phi[:])
            lo = wp.tile([P, 1], f32, tag="lo")
            nc.vector.memset(lo[:], 0.0)
            tstar = wp.tile([P, 1], f32, tag="tstar")
            nc.vector.memset(tstar[:], 0.0)
            sgn = wp.tile([P, FLAT], f32, tag="sgn")
            BIG = 1.0e30

            def preduce(vec_ap, op, name):
                # [P,1] -> scalar [1,1] via transpose + free reduce
                pt_ = pp_tp.tile([1, P], f32, tag="tp", name=f"pt_{name}")
                nc.tensor.transpose(pt_[:], vec_ap, idn[:P, :P])
                row = tp.tile([1, P], f32, tag="prow", name="prow")
                nc.vector.tensor_copy(row[:], pt_[:])
                sc_ = tp.tile([1, 1], f32, tag="pscl", name="pscl")
                nc.vector.tensor_reduce(sc_[:], row[:], axis=AX.X, op=op)
                return sc_

            def bcast_col(scalar_ap, name):
                pb_ = pp_tp.tile([P, 1], f32, tag="tp", name=f"pb_{name}")
                nc.tensor.matmul(pb_[:], onr[:1, :P], scalar_ap, start=True,
                                 stop=True)
                return pb_

            for rnd in range(2):
                # probes t_i = lo + (i+1)*(hi-lo)/63
                stp = tp.tile([P, 1], f32, tag="stp", name="stp")
                nc.vector.tensor_sub(stp[:], hi[:], lo[:])
                nc.vector.tensor_scalar(stp[:], stp[:], 1.0 / 63.0, None,
                                        OP.mult)
                tcol = tp.tile([P, 1], f32, tag="tcol", name="tcol")
                nc.vector.scalar_tensor_tensor(tcol[:], ioc1[:], stp[:],
                                               lo[:], OP.mult, OP.add)
                nbt = tp.tile([P, 1], f32, tag="nbt", name="nbt")
                nc.vector.tensor_scalar(nbt[:], tcol[:], -1.0, None, OP.mult)
                ssg = tp.tile([P, 1], f32, tag="ssg", name="ssg")
                nc.scalar.activation(sgn[:], R[:], AF.Sign, bias=nbt[:],
                                     accum_out=ssg[:])
                cnt = tp.tile([P, 1], f32, tag="cnt", name="cnt")
                nc.vector.tensor_scalar(cnt[:], ssg[:], 0.5, FLAT / 2.0,
                                        OP.mult, OP.add)
                # candidate columns: [lo-cand, -hi-cand, t*-cand]
                cand = tp.tile([P, 4], f32, tag="cand", name="cand")
                mlo = tp.tile([P, 1], f32, tag="mlo", name="mlo")
                nc.vector.tensor_scalar(mlo[:], cnt[:], 62.75, None, OP.is_ge)
                nc.vector.tensor_mul(cand[:, 0:1], tcol[:], mlo[:])
                mhi = tp.tile([P, 1], f32, tag="mhi", name="mhi")
                nc.vector.tensor_scalar(mhi[:], cnt[:], 62.25, None, OP.is_le)
                hc = tp.tile([P, 1], f32, tag="hc", name="hc")
                nc.vector.tensor_mul(hc[:], tcol[:], mhi[:])
                hc2 = tp.tile([P, 1], f32, tag="hc2", name="hc2")
                nc.vector.tensor_scalar(hc2[:], mhi[:], BIG, -BIG, OP.mult,
                                        OP.add)
                # cand1 = -(t*mhi + BIG*(1-mhi)) = hc2 - hc   (hc2 above is
                # mhi*BIG - BIG = -(BIG*(1-mhi)))
                nc.vector.tensor_sub(cand[:, 1:2], hc2[:], hc[:])
                c62 = tp.tile([P, 1], f32, tag="c62", name="c62")
                nc.vector.tensor_scalar(c62[:], cnt[:], -62.0, None, OP.add)
                sq = tp.tile([P, 1], f32, tag="sq", name="sq")
                nc.vector.tensor_mul(sq[:], c62[:], c62[:])
                meq = tp.tile([P, 1], f32, tag="meq", name="meq")
                nc.vector.tensor_scalar(meq[:], sq[:], 0.07, None, OP.is_le)
                nc.vector.tensor_mul(cand[:, 2:3], tcol[:], meq[:])
                nc.vector.memset(cand[:, 3:4], 0.0)
                # one transpose + one max-reduce handles all three updates
                pcd = pp_tp.tile([4, P], f32, tag="tp", name=f"pcd{rnd}")
                nc.tensor.transpose(pcd[:], cand[:], idn[:P, :P])
                cdr = tp.tile([4, P], f32, tag="cdr", name="cdr")
                nc.vector.tensor_copy(cdr[:], pcd[:])
                mx3 = tp.tile([4, 1], f32, tag="mx3", name="mx3")
                nc.vector.tensor_reduce(mx3[:], cdr[:], axis=AX.X, op=OP.max)
                mx3r = tp.tile([1, 4], f32, tag="mx3r", name="mx3r")
                pmx3 = pp_tp.tile([1, 4], f32, tag="tp", name=f"pmx3{rnd}")
                nc.tensor.transpose(pmx3[:], mx3[:], idn[:4, :4])
                nc.vector.tensor_copy(mx3r[:], pmx3[:])
                pbc3 = pp_tp.tile([P, 4], f32, tag="tp", name=f"pbc3{rnd}")
                nc.tensor.matmul(pbc3[:], onr[:1, :P], mx3r[:1, :],
                                 start=True, stop=True)
                nc.vector.tensor_max(lo[:], lo[:], pbc3[:, 0:1])
                nhi = tp.tile([P, 1], f32, tag="nhi", name="nhi")
                nc.vector.tensor_scalar(nhi[:], pbc3[:, 1:2], -1.0, None,
                                        OP.mult)
                nc.vector.tensor_tensor(hi[:], hi[:], nhi[:], OP.min)
                nc.vector.tensor_max(tstar[:], tstar[:], pbc3[:, 2:3])
            # (3) mask / compaction ranks / one-hots (verified scheme)
            mask = wp.tile([P, P], f32, tag="mask")
            nc.vector.tensor_scalar(mask[:], PA[:], tstar[:], None, OP.is_gt)
            pmT = pp_sel.tile([P, P], f32, tag="ps", name="pmT")
            nc.tensor.transpose(pmT[:], mask[:], idn[:P, :P])
            mT = wp.tile([P, P], f32, tag="mT")
            nc.vector.tensor_copy(mT[:], pmT[:])
            prc = pp_sel.tile([P, P], f32, tag="ps", name="prc")
            nc.tensor.matmul(prc[:], mT[:], lt[:], start=True, stop=True)
            rcm = wp.tile([P, P], f32, tag="rcm")
            nc.vector.tensor_copy(rcm[:], prc[:])
            pro = pp_tp.tile([1, P], f32, tag="tp", name="pro")
            nc.tensor.matmul(pro[:], rcm[:, P - 1:P], slt[:], start=True,
                             stop=True)
            ror = wp.tile([1, P], f32, tag="ror")
            nc.vector.tensor_copy(ror[:], pro[:])
            proc = pp_tp.tile([P, 1], f32, tag="tp", name="proc")
            nc.tensor.transpose(proc[:], ror[:1, :], idn[:1, :1])
            roc = wp.tile([P, 1], f32, tag="roc")
            nc.vector.tensor_copy(roc[:], proc[:])
            re_ = wp.tile([P, 1], f32, tag="re")
            nc.vector.tensor_add(re_[:], roc[:], rcm[:, P - 1:P])
            g1 = tp.tile([P, P], f32, tag="selu_m", name="g1")
            nc.vector.tensor_scalar(g1[:], iof[:], roc[:], None, OP.is_ge)
            g2 = tp.tile([P, P], f32, tag="selu_r", name="g2")
            nc.vector.tensor_scalar(g2[:], iof[:], re_[:], None, OP.is_lt)
            bb = wp.tile([P, P], f32, tag="bb")
            nc.vector.tensor_mul(bb[:], g1[:], g2[:])
            t1 = tp.tile([P, P], f32, tag="selu_e", name="t1")
            nc.vector.tensor_scalar(t1[:], rcm[:], roc[:], None, OP.add)
            t2 = tp.tile([P, P], f32, tag="selu_e2", name="t2")
            nc.vector.tensor_mul(t2[:], t1[:], mask[:])
            t3 = tp.tile([P, P], f32, tag="selu_m", name="t3")
            nc.vector.tensor_scalar(t3[:], t2[:], -1.0, None, OP.add)
            idx = wp.tile([64, P], i16, tag="idx")
            nc.vector.memset(idx[:], -1)
            nc.vector.tensor_copy(idx[:P, :], t3[:])
            scx = wp.tile([64, 64], bf16, tag="scx")
            nc.gpsimd.local_scatter(scx[:], iofb[:], idx[:], channels=64,
                                    num_elems=64, num_idxs=P)
            pas = pp_tp.tile([1, P], f32, tag="tp", name="pas")
            nc.tensor.matmul(pas[:], ocb[:], scx[:P, :P], start=True,
                             stop=True)
            asr = wp.tile([1, P], f32, tag="asr")
            nc.vector.tensor_copy(asr[:], pas[:])
            pab = pp_sel.tile([P, P], f32, tag="ps", name="pab")
            nc.tensor.matmul(pab[:], onr[:1, :P], asr[:1, :], start=True,
                             stop=True)
            ba = wp.tile([P, P], f32, tag="ba")
            nc.vector.tensor_scalar(ba[:], pab[:], ioc[:], None, OP.is_equal)
            pz = pp_sel.tile([P, P], f32, tag="ps", name="pz")
            nc.tensor.matmul(pz[:], bb[:], PA[:], start=True, stop=True)
            pbat = pp_sel.tile([P, P], f32, tag="ps", name="pbat")
            nc.tensor.transpose(pbat[:], ba[:], idn[:P, :P])
            bat = wp.tile([P, P], f32, tag="bat")
            nc.vector.tensor_copy(bat[:], pbat[:])
            pmm = tp.tile([P, P], f32, tag="selu_r", name="pmm")
            nc.vector.tensor_mul(pmm[:], pz[:], bat[:])
            pcol = wp.tile([P, 1], f32, tag="pcol")
            nc.vector.tensor_reduce(pcol[:], pmm[:], axis=AX.X, op=OP.add)

            # ---------------- stage 3 (tenor) ----------------
            h1t = []
            for mt in range(MT):
                pg = pp_l1.tile([128, P], f32, tag="l2", name="pg")
                nc.tensor.matmul(pg[:], tohb[:, mt * 128:(mt + 1) * 128],
                                 bb[:], start=True, stop=False)
                nc.tensor.matmul(pg[:], toha[:, mt * 128:(mt + 1) * 128],
                                 ba[:], start=False, stop=True)
                h1t.append(selu_chain(pg[:], sht[mt][:], 128, P, f"h1t{mt}"))

            ht1 = dp.tile([HS, P], f32, tag="ht1")
            for mt in range(MT):
                nc.gpsimd.dma_start(ht1[mt * 128:(mt + 1) * 128, :],
                                    h1t[mt][:])
            ght = dp.tile([H, P], f32, tag="ght")
            nc.gpsimd.collective_compute(
                "AllGather", OP.bypass, replica_groups=RG,
                ins=[ht1[:].opt()], outs=[ght[:].opt()])
            H1tT = wp.tile([128, KT2 * P], f32, tag="HT", bufs=2, name="H1tT")
            nc.sync.dma_start(
                H1tT[:].rearrange("p (kt w) -> p kt w", w=P),
                ght[:].rearrange("(kt p) w -> p kt w", p=128))

            h2t = []
            for mt in range(MT):
                pyt = pp_l1.tile([128, P], f32, tag="l2", name="pyt")
                for kt in range(KT2):
                    nc.tensor.matmul(
                        pyt[:],
                        w2s["t"][:, kt * HS + mt * 128:kt * HS + (mt + 1) * 128],
                        H1tT[:, kt * P:(kt + 1) * P],
                        start=(kt == 0), stop=False)
                nc.tensor.matmul(pyt[:], b2s["t"][:1, mt * 128:(mt + 1) * 128],
                                 onr[:1, :P], start=False, stop=True)
                h2t.append(selu_chain(pyt[:], None, 128, P, f"h2t{mt}"))

            ht2 = dp.tile([HS, P], f32, tag="ht2")
            for mt in range(MT):
                nc.gpsimd.dma_start(ht2[mt * 128:(mt + 1) * 128, :],
                                    h2t[mt][:])
            ght2 = dp.tile([H, P], f32, tag="ght2")
            nc.gpsimd.collective_compute(
                "AllGather", OP.bypass, replica_groups=RG,
                ins=[ht2[:].opt()], outs=[ght2[:].opt()])
            H2tT = wp.tile([128, KT2 * P], f32, tag="HT", bufs=2, name="H2tT")
            nc.sync.dma_start(
                H2tT[:].rearrange("p (kt w) -> p kt w", w=P),
                ght2[:].rearrange("(kt p) w -> p kt w", p=128))

            plg_t = pp_sel.tile([P, P], f32, tag="ps", name="plg_t")
            for kt in range(KT2):
                nc.tensor.matmul(plg_t[:], H2tT[:, kt * P:(kt + 1) * P],
                                 w3s["t"][:, kt * P:(kt + 1) * P],
                                 start=(kt == 0), stop=False)
            nc.tensor.matmul(plg_t[:], onr[:1, :P], b3s["t"][:1, :],
                             start=False, stop=True)
            S3 = wp.tile([P, P], f32, tag="S3")
            nc.vector.tensor_copy(S3[:], plg_t[:])
            nm3 = wp.tile([P, 1], f32, tag="nm3")
            nc.vector.tensor_reduce(nm3[:], S3[:], axis=AX.X, op=OP.max,
                                    negate=True)
            E3 = wp.tile([P, P], f32, tag="E3")
            ssum3 = wp.tile([P, 1], f32, tag="ssum3")
            nc.scalar.activation(E3[:], S3[:], AF.Exp, bias=nm3[:],
                                 accum_out=ssum3[:])
            rec3 = wp.tile([P, 1], f32, tag="rec3")
            nc.vector.reciprocal(rec3[:], ssum3[:])
            vv = wp.tile([P, 1], f32, tag="vv")
            nc.vector.tensor_mul(vv[:], rec3[:], pcol[:])
            PT = wp.tile([P, P], f32, tag="PT")
            nc.vector.tensor_scalar(PT[:], E3[:], vv[:], None, OP.mult)
            nc.scalar.dma_start(pt_out[:], PT[:])

    nc.compile()
    return nc


_NC_CACHE = None


def _get_nc():
    global _NC_CACHE
    if _NC_CACHE is None:
        _NC_CACHE = _build()
    return _NC_CACHE


def _prep_inputs(inputs):
    lam = np.float32(LAM)
    x = np.asarray(inputs["inputs_bass"], np.float32)

    def w1timg(w):
        # [D, 256] -> transposed image [128, MT*D]: img[p, mt*D+k] = w[k, mt*128+p]
        wt = np.ascontiguousarray(w.T)              # [256, D]
        return np.ascontiguousarray(
            wt.reshape(MT, 128, D).transpose(1, 0, 2).reshape(128, MT * D))

    def w2img(w):
        return np.ascontiguousarray(
            w.reshape(KT2, 128, HS).transpose(1, 0, 2).reshape(128, KT2 * HS))

    def w3img(w):
        # [2048, P] -> [128, KT2*P]
        return np.ascontiguousarray(
            w.reshape(KT2, 128, P).transpose(1, 0, 2).reshape(128, KT2 * P))

    def mtimg(w):
        # [256, P] -> [128, MT*P]
        return np.ascontiguousarray(
            w.reshape(MT, 128, P).transpose(1, 0, 2).reshape(128, MT * P))

    base = {
        "ident": np.eye(128, dtype=np.float32),
        "LTc": (np.arange(P)[:, None] <= np.arange(P)[None, :]).astype(np.float32),
        "SLTc": (np.arange(P)[:, None] < np.arange(P)[None, :]).astype(np.float32),
        "iotaF": np.broadcast_to(np.arange(P, dtype=np.float32), (P, P)).copy(),
        "iotaC": np.arange(P, dtype=np.float32)[:, None].copy(),
        "iotaC1": (np.arange(P, dtype=np.float32)[:, None] + 1.0).copy(),
        "onesR": np.ones((1, HS), np.float32),
        "onesCbf": np.ones((P, 1), ml_dtypes.bfloat16),
        "iotaFbf": np.broadcast_to(
            np.arange(P, dtype=ml_dtypes.bfloat16), (64, P)).copy(),
        "xbi": np.ascontiguousarray(np.broadcast_to(x, (128, D))),
    }
    W = {k: np.asarray(v, np.float32) for k, v in inputs.items()}
    in_maps = []
    for c in range(NCORES):
        cols = slice(HS * c, HS * (c + 1))
        m = dict(base)
        for s in "bat":
            m[f"{s}w1t"] = w1timg(lam * W[f"{s}w1"][:D, cols])
            m[f"{s}w2i"] = w2img(lam * W[f"{s}w2"][:, cols])
            m[f"{s}w3i"] = w3img(W[f"{s}w3"])
            m[f"{s}b1c"] = np.ascontiguousarray(
                (lam * W[f"{s}b1"][cols]).reshape(MT, 128).T)
            m[f"{s}b2r"] = (lam * W[f"{s}b2"][cols])[None, :].copy()
            m[f"{s}b3r"] = W[f"{s}b3"][None, :].copy()
        m["aohT"] = mtimg(np.ascontiguousarray(
            (lam * W["aw1"][D:D + P, cols]).T))
        m["tohb"] = np.ascontiguousarray(lam * W["tw1"][D:D + P, cols])
        m["toha"] = np.ascontiguousarray(lam * W["tw1"][D + P:D + 2 * P, cols])
        in_maps.append(m)
    return in_maps


def _postprocess(pa, pt):
    flat = pa.reshape(-1)
    order = np.argsort(-flat, kind="stable")[:P]
    sel = np.sort(order)                  # device rank order = flat position
    j_sel = sel // P
    a_sel = sel % P
    flat3 = pt.reshape(-1)
    idx3 = np.argsort(-flat3, kind="stable")[:P]
    row = idx3 // P
    out = np.stack([
        flat3[idx3],
        j_sel[row].astype(np.float32),
        a_sel[row].astype(np.float32),
        (idx3 % P).astype(np.float32),
    ], axis=1)
    return out


def run(inputs, trace=False):
    nc = _get_nc()
    in_maps = _prep_inputs(inputs)
    res = bass_utils.run_bass_kernel_spmd(
        nc, in_maps, core_ids=list(range(NCORES)), trace=trace)
    r0 = res.results[0]
    out = _postprocess(r0["pa_out"], r0["pt_out"])
    return out, res.exec_time_ns


def kernel(**inputs) -> np.ndarray:
    out, _ = run(inputs, trace=False)
    return out


# revision 20
# speedup vs baseline: 1.1169x; 1.0807x over previous
"""BachNet beam-search inference kernel for 8 TRN2 NeuronCores.

Strategy (single NEFF launch, tensor-parallel over the hidden dim):
  - N == P == 62, so stage-1's sort only reorders rows; stages are computed in
    natural pitch order and the one-hot concatenations become row-slices /
    row-gathers of the first-layer weight matrices.
  - Each core owns a 256-wide column shard of every w1/w2; w3 is replicated.
    The x @ w1 mat-vecs run on VectorE as fused multiply-reduce over
    transposed weight images; the batched layer-2 GEMMs run on TensorE.
    One AllGather shares layer-1 activations, a second shares layer-2
    activations (logits are then computed locally from replicated w3).
  - The stage-2 top-62 selection runs fully on-device and replicated: a
    3-round 62-ary probe search (ScalarE sign-count against a broadcast
    copy of the flattened scores) finds a threshold with exactly 62
    elements above it; triangular matmuls turn the mask into row-major
    compaction ranks, and a gpsimd local_scatter builds the alto one-hot.
  - The final (stage-3) top-62 + sort runs on host from the tiny [62,62]
    result matrices (exact, matches jnp.argsort tie-breaking).
  - selu is computed as lam*relu(v) + lam*alpha*(exp(min(v,0))-1) with the
    lam factor pre-folded into the layer-1/2 weights on host.
"""
import sys

sys.path.insert(0, "/opt/trn_rl_repo")

import numpy as np
import ml_dtypes

import concourse.bacc as bacc
import concourse.tile as tile
import concourse.mybir as mybir
from concourse import bass_utils

P = 62           # pitch classes == num candidates
D = 10112        # bass input dim (= 79 * 128)
H = 2048         # hidden
NCORES = 8
HS = H // NCORES          # 256 hidden columns per core
KT1 = D // 128            # 79 k-tiles for layer 1
KT2 = H // 128            # 16 k-tiles for layer 2
MT = HS // 128            # 2 m-tiles per core shard
CHK = 2528                # layer-1 k-chunk (D = 4*2528)
NCH = D // CHK            # 4 chunks per h-tile
LAM = 1.0507009873554805
ALPHA = 1.6732632423543772
LA = LAM * ALPHA
FLAT = P * P              # 3844

f32 = mybir.dt.float32
bf16 = mybir.dt.bfloat16
i16 = mybir.dt.int16
OP = mybir.AluOpType
AX = mybir.AxisListType
AF = mybir.ActivationFunctionType
RG = [list(range(NCORES))]


def _build():
    nc = bacc.Bacc("TRN2", target_bir_lowering=False, debug=False,
                   num_devices=NCORES)

    def din(name, shape, dtype=f32):
        return nc.dram_tensor(name, shape, dtype, kind="ExternalInput")

    xT_d = din("xT", [128, KT1])
    w1_d = {s: din(f"{s}w1i", [128, KT1 * HS]) for s in "bat"}
    w2_d = {s: din(f"{s}w2i", [128, KT2 * HS]) for s in "bat"}
    w3_d = {s: din(f"{s}w3i", [128, KT2 * P]) for s in "bat"}
    aohT_d = din("aohT", [128, MT * P])
    tohb_d = din("tohb", [P, HS])
    toha_d = din("toha", [P, HS])
    b1_d = {s: din(f"{s}b1r", [1, HS]) for s in "bat"}
    one_d = din("one1", [1, 1])
    b2_d = {s: din(f"{s}b2r", [1, HS]) for s in "bat"}
    b3_d = {s: din(f"{s}b3r", [1, P]) for s in "bat"}
    ident_d = din("ident", [128, 128])
    LT_d = din("LTc", [P, P])
    SLT_d = din("SLTc", [P, P])
    iotaF_d = din("iotaF", [P, P])
    iotaC_d = din("iotaC", [P, 1])
    iotaC1_d = din("iotaC1", [P, 1])
    onesR_d = din("onesR", [1, HS])
    onesCbf_d = din("onesCbf", [P, 1], bf16)
    iotaFbf_d = din("iotaFbf", [64, P], bf16)

    pa_out = nc.dram_tensor("pa_out", [P, P], f32, kind="ExternalOutput")
    pt_out = nc.dram_tensor("pt_out", [P, P], f32, kind="ExternalOutput")

    with tile.TileContext(nc) as tc:
        with (
            tc.tile_pool(name="consts", bufs=1) as cp,
            tc.tile_pool(name="stream", bufs=4) as sp,
            tc.tile_pool(name="work", bufs=1) as wp,
            tc.tile_pool(name="trans", bufs=3) as tp,
            tc.tile_pool(name="pmv", bufs=1, space="PSUM") as pp_mv,
            tc.tile_pool(name="ptp", bufs=2, space="PSUM") as pp_tp,
            tc.tile_pool(name="pl1", bufs=2, space="PSUM") as pp_l1,
            tc.tile_pool(name="psel", bufs=2, space="PSUM") as pp_sel,
            tc.tile_pool(name="dram", bufs=1, space="DRAM") as dp,
        ):
            def cload(src, shape, dtype=f32, eng=None):
                t = cp.tile(shape, dtype, tag=src.name, name="c_" + src.name)
                (eng or nc.sync).dma_start(t[:], src[:])
                return t

            # --- small constants (sync queue, ahead of the weight stream) ---
            xTs = cload(xT_d, [128, KT1])
            idn = cload(ident_d, [128, 128])
            lt = cload(LT_d, [P, P])
            slt = cload(SLT_d, [P, P])
            iof = cload(iotaF_d, [P, P])
            ioc = cload(iotaC_d, [P, 1])
            ioc1 = cload(iotaC1_d, [P, 1])
            onr = cload(onesR_d, [1, HS])
            ocb = cload(onesCbf_d, [P, 1], bf16)
            iofb = cload(iotaFbf_d, [64, P], bf16)
            b1s = {s: cload(b1_d[s], [1, HS]) for s in "bat"}
            one1 = cload(one_d, [1, 1])
            b1r = {s: one1 for s in "bat"}
            b2s = {s: cload(b2_d[s], [1, HS]) for s in "bat"}
            b3s = {s: cload(b3_d[s], [1, P]) for s in "bat"}

            # --- warmup collective: trigger ASAP on uninitialized dram ---
            wbi = dp.tile([16, 32], f32, tag="wbi")
            wbo = dp.tile([128, 32], f32, tag="wbo")
            nc.gpsimd.collective_compute(
                "AllGather", OP.bypass, replica_groups=RG,
                ins=[wbi[:].opt()], outs=[wbo[:].opt()])
            # readback on ScalarE's queue so it doesn't head-of-line block
            # the gpsimd DMA stream while the collective stack initializes


            aohT = cload(aohT_d, [128, MT * P])
            tohb = cload(tohb_d, [P, HS])
            toha = cload(toha_d, [P, HS])


            # --- layer-1 mat-vec on TensorE (x columns stationary):
            # psh[0, m] = sum_k x[k] * w1[k, m], chunk-streamed from DRAM ---
            _CH10 = []
            _t = 0
            while _t < KT1:
                _n = min(10, KT1 - _t)
                _CH10.append((_t, _n))
                _t += _n

            def matvec(s):
                psh = pp_mv.tile([1, HS], f32, tag="mv", name=f"psh_{s}")
                for (t0, tn) in _CH10:
                    ck = sp.tile([128, 10 * HS], f32, tag="w1ck", name="w1ck")
                    nc.sync.dma_start(
                        ck[:, :tn * HS],
                        w1_d[s][:, t0 * HS:(t0 + tn) * HS])
                    for t in range(tn):
                        nc.tensor.matmul(psh[:], xTs[:, t0 + t:t0 + t + 1],
                                         ck[:, t * HS:(t + 1) * HS],
                                         start=(t0 + t == 0), stop=False)
                nc.tensor.matmul(psh[:], b1r[s][:1, 0:1], b1s[s][:1, :],
                                 start=False, stop=True)
                shrow = tp.tile([1, HS], f32, tag="shrow", name="shrow")
                nc.vector.tensor_copy(shrow[:], psh[:])
                cols = []
                for mt in range(MT):
                    ptpm = pp_tp.tile([128, 1], f32, tag="tp", name="ptpm")
                    nc.tensor.transpose(ptpm[:],
                                        shrow[:1, mt * 128:(mt + 1) * 128],
                                        idn[:1, :1])
                    scol = wp.tile([128, 1], f32, tag=f"shc_{s}{mt}",
                                   name=f"shc_{s}{mt}")
                    nc.vector.tensor_copy(scol[:], ptpm[:])
                    cols.append(scol)
                return cols

            # selu: dst = lam*relu(pre) + lam*alpha*(exp(min(pre,0))-1)
            def selu_chain(pre_ap, shcol, parts, width, tag):
                shp = [parts, width]
                m = tp.tile(shp, f32, tag="selu_m", name="selu_m")
                r = tp.tile(shp, f32, tag="selu_r", name="selu_r")
                e = tp.tile(shp, f32, tag="selu_e", name="selu_e")
                e2 = tp.tile(shp, f32, tag="selu_e2", name="selu_e2")
                dst = wp.tile(shp, f32, tag=tag, name=tag)
                if shcol is None:
                    nc.vector.tensor_scalar(m[:], pre_ap, 0.0, None, OP.min)
                    nc.vector.tensor_scalar(r[:], pre_ap, 0.0, None, OP.max)
                else:
                    nc.vector.tensor_scalar(m[:], pre_ap, shcol, 0.0, OP.add,
                                            OP.min)
                    nc.vector.tensor_scalar(r[:], pre_ap, shcol, 0.0, OP.add,
                                            OP.max)
                nc.scalar.activation(e[:], m[:], AF.Exp, scale=1.0 / LAM)
                nc.vector.tensor_scalar(e2[:], e[:], LA, -LA, OP.mult, OP.add)
                nc.vector.tensor_add(dst[:], r[:], e2[:])
                return dst

            # ---------------- stage 1+2 layer 1 (bass || alto) ----------
            W1 = P + 1
            sha = matvec("a")
            w2s = {}
            w3s = {}
            w2s["a"] = cload(w2_d["a"], [128, KT2 * HS])
            w3s["a"] = cload(w3_d["a"], [128, KT2 * P])
            h1a = [selu_chain(aohT[:, mt * P:(mt + 1) * P], sha[mt][:], 128, P,
                              f"h1a{mt}")
                   for mt in range(MT)]
            # AllGather the alto activations alone: they gate the whole chain,
            # while the bass vector joins later (only needed at the PA stage)
            hb1 = dp.tile([HS, P], f32, tag="hb1")
            for mt in range(MT):
                nc.gpsimd.dma_start(hb1[mt * 128:(mt + 1) * 128, :],
                                    h1a[mt][:])
            ghb1 = dp.tile([H, P], f32, tag="ghb1")
            nc.gpsimd.collective_compute(
                "AllGather", OP.bypass, replica_groups=RG,
                ins=[hb1[:].opt()], outs=[ghb1[:].opt()])
            H1aT = wp.tile([128, KT2 * P], f32, tag="HT", bufs=2, name="H1aT")
            for q in range(4):
                nc.gpsimd.dma_start(
                    H1aT[:, q * 4 * P:(q + 1) * 4 * P].rearrange(
                        "p (kt w) -> p kt w", w=P),
                    ghb1[q * 512:(q + 1) * 512, :].rearrange(
                        "(kt p) w -> p kt w", p=128))
            wg = wp.tile([128, 32], f32, tag="warm2")
            nc.gpsimd.dma_start(wg[:], wbo[:])

            shb = matvec("b")
            w2s["b"] = cload(w2_d["b"], [128, KT2 * HS])
            w3s["b"] = cload(w3_d["b"], [128, KT2 * P])
            h1b = [selu_chain(shb[mt][:], None, 128, 1, f"h1b{mt}")
                   for mt in range(MT)]
            hb1b = dp.tile([HS, 1], f32, tag="hb1b")
            for mt in range(MT):
                nc.gpsimd.dma_start(hb1b[mt * 128:(mt + 1) * 128, :],
                                    h1b[mt][:])
            ghb1b = dp.tile([H, 1], f32, tag="ghb1b")
            nc.gpsimd.collective_compute(
                "AllGather", OP.bypass, replica_groups=RG,
                ins=[hb1b[:].opt()], outs=[ghb1b[:].opt()])
            H1bT = wp.tile([128, KT2], f32, tag="H1bT")
            nc.gpsimd.dma_start(
                H1bT[:],
                ghb1b[:].rearrange("(kt p) w -> p (kt w)", p=128))

            # ------------- stage 3 layer-1 mat-vec (independent) ---------
            sht = matvec("t")
            w2s["t"] = cp.tile([128, KT2 * HS], f32, tag="tw2i", name="c_tw2i")
            nc.sync.dma_start(w2s["t"][:], w2_d["t"][:])
            w3s["t"] = cp.tile([128, KT2 * P], f32, tag="tw3i", name="c_tw3i")
            nc.sync.dma_start(w3s["t"][:], w3_d["t"][:])

            # ---------------- stage 1+2 layer 2 + logits ----------------
            h2a = []
            h2b = []
            for mt in range(MT):
                pya = pp_l1.tile([128, P], f32, tag="l2", name="pya")
                for kt in range(KT2):
                    nc.tensor.matmul(
                        pya[:],
                        w2s["a"][:, kt * HS + mt * 128:kt * HS + (mt + 1) * 128],
                        H1aT[:, kt * P:(kt + 1) * P],
                        start=(kt == 0), stop=False)
                nc.tensor.matmul(pya[:], b2s["a"][:1, mt * 128:(mt + 1) * 128],
                                 onr[:1, :P], start=False, stop=True)
                h2a.append(selu_chain(pya[:], None, 128, P, f"h2a{mt}"))
                pyb = pp_tp.tile([128, 1], f32, tag="tp", name="pyb")
                for kt in range(KT2):
                    nc.tensor.matmul(
                        pyb[:],
                        w2s["b"][:, kt * HS + mt * 128:kt * HS + (mt + 1) * 128],
                        H1bT[:, kt:kt + 1],
                        start=(kt == 0), stop=False)
                nc.tensor.matmul(pyb[:], b2s["b"][:1, mt * 128:(mt + 1) * 128],
                                 onr[:1, :1], start=False, stop=True)
                h2b.append(selu_chain(pyb[:], None, 128, 1, f"h2b{mt}"))

            # AllGather h2 (fused alto+bass), then local logits vs full w3
            hb2 = dp.tile([HS, W1], f32, tag="hb2")
            for mt in range(MT):
                nc.gpsimd.dma_start(hb2[mt * 128:(mt + 1) * 128, 0:P],
                                    h2a[mt][:])
                nc.gpsimd.dma_start(hb2[mt * 128:(mt + 1) * 128, P:W1],
                                    h2b[mt][:])
            ghb2 = dp.tile([H, W1], f32, tag="ghb2")
            nc.gpsimd.collective_compute(
                "AllGather", OP.bypass, replica_groups=RG,
                ins=[hb2[:].opt()], outs=[ghb2[:].opt()])
            H2T = wp.tile([128, KT2 * W1], f32, tag="HT", bufs=2, name="H2T")
            nc.sync.dma_start(
                H2T[:].rearrange("p (kt w) -> p kt w", w=W1),
                ghb2[:].rearrange("(kt p) w -> p kt w", p=128))

            plg_a = pp_sel.tile([P, P], f32, tag="ps", name="plg_a")
            for kt in range(KT2):
                nc.tensor.matmul(plg_a[:], H2T[:, kt * W1:kt * W1 + P],
                                 w3s["a"][:, kt * P:(kt + 1) * P],
                                 start=(kt == 0), stop=False)
            nc.tensor.matmul(plg_a[:], onr[:1, :P], b3s["a"][:1, :],
                             start=False, stop=True)
            plg_b = pp_tp.tile([1, P], f32, tag="tp", name="plg_b")
            for kt in range(KT2):
                nc.tensor.matmul(plg_b[:], H2T[:, kt * W1 + P:kt * W1 + W1],
                                 w3s["b"][:, kt * P:(kt + 1) * P],
                                 start=(kt == 0), stop=False)
            nc.tensor.matmul(plg_b[:], onr[:1, :1], b3s["b"][:1, :],
                             start=False, stop=True)

            # fused softmax: alto rows 0..61 at base 0, bass row copied to 64
            NR = 65
            lgcat = wp.tile([NR, P], f32, tag="lgcat")
            nc.vector.memset(lgcat[:], 0.0)
            nc.vector.tensor_copy(lgcat[:P, :], plg_a[:])
            nc.vector.tensor_copy(lgcat[64:NR, :], plg_b[:])
            nm = wp.tile([NR, 1], f32, tag="nm")
            nc.vector.tensor_reduce(nm[:], lgcat[:], axis=AX.X, op=OP.max,
                                    negate=True)
            E = wp.tile([NR, P], f32, tag="E")
            ssum = wp.tile([NR, 1], f32, tag="ssum")
            nc.scalar.activation(E[:], lgcat[:], AF.Exp, bias=nm[:],
                                 accum_out=ssum[:])
            rec = wp.tile([NR, 1], f32, tag="rec")
            nc.vector.reciprocal(rec[:], ssum[:])
            erow = wp.tile([1, P], f32, tag="erow")
            nc.vector.tensor_copy(erow[:], E[64:NR, :])
            rc62 = wp.tile([1, 1], f32, tag="rc62")
            nc.vector.tensor_copy(rc62[:], rec[64:NR, 0:1])
            ptp2 = pp_tp.tile([P, 1], f32, tag="tp", name="ptp2")
            nc.tensor.transpose(ptp2[:], erow[:1, :], idn[:1, :1])
            pbc = pp_tp.tile([P, 1], f32, tag="tp", name="pbc")
            nc.tensor.matmul(pbc[:], onr[:1, :P], rc62[:1, :1],
                             start=True, stop=True)
            v1 = wp.tile([P, 1], f32, tag="v1")
            nc.vector.tensor_mul(v1[:], ptp2[:], rec[:P, :])
            v = wp.tile([P, 1], f32, tag="v")
            nc.vector.tensor_mul(v[:], v1[:], pbc[:])
            # anchor the warmup collective so it isn't dead code
            # (wbo is uninitialized garbage: is_ge maps any bits, incl. NaN,
            # to 0/1, and the huge threshold makes the result 0)
            wanc = wp.tile([P, 1], f32, tag="wanc")
            nc.vector.tensor_scalar(wanc[:], wg[:P, 0:1], 1e38, None, OP.is_ge)
            nc.vector.scalar_tensor_tensor(v[:], wanc[:], 0.0, v[:],
                                           OP.mult, OP.add)
            PA = wp.tile([P, P], f32, tag="PA")
            nc.vector.tensor_scalar(PA[:], E[:P, :], v[:], None, OP.mult)
            nc.scalar.dma_start(pa_out[:], PA[:])

            # ---------------- on-device top-62 selection ----------------
            # (1) broadcast flat scores to all partitions: R[i, e] = PA_flat[e]
            paf = dp.tile([P, P], f32, tag="paf")
            nc.gpsimd.dma_start(paf[:], PA[:])
            R = wp.tile([P, FLAT], f32, tag="R")
            nc.gpsimd.dma_start(
                R[:],
                paf[:].rearrange("a b -> (a b)")[None, :].broadcast_to(
                    [P, FLAT]))
            # (2) initial bracket: lo = 0, hi = max * 1.00001
            rmx = wp.tile([P, 1], f32, tag="rmx")
            nc.vector.tensor_reduce(rmx[:], PA[:], axis=AX.X, op=OP.max)
            prx = pp_tp.tile([1, P], f32, tag="tp", name="prx")
            nc.tensor.transpose(prx[:], rmx[:], idn[:P, :P])
            rxr = wp.tile([1, P], f32, tag="rxr")
            nc.vector.tensor_copy(rxr[:], prx[:])
            vmx = wp.tile([1, 1], f32, tag="vmx")
            nc.vector.tensor_reduce(vmx[:], rxr[:], axis=AX.X, op=OP.max)
            nc.vector.tensor_scalar(vmx[:], vmx[:], 1.00001, None, OP.mult)
            phi = pp_tp.tile([P, 1], f32, tag="tp", name="phi")
            nc.tensor.matmul(phi[:], onr[:1, :P], vmx[:1, :1], start=True,
                             stop=True)
            hi = wp.tile([P, 1], f32, tag="hi")
            nc.vector.tensor_copy(hi[:], phi[:])
            lo = wp.tile([P, 1], f32, tag="lo")
            nc.vector.memset(lo[:], 0.0)
            tstar = wp.tile([P, 1], f32, tag="tstar")
            nc.vector.memset(tstar[:], 0.0)
            sgn = wp.tile([P, FLAT], f32, tag="sgn")
            BIG = 1.0e30

            def preduce(vec_ap, op, name):
                # [P,1] -> scalar [1,1] via transpose + free reduce
                pt_ = pp_tp.tile([1, P], f32, tag="tp", name=f"pt_{name}")
                nc.tensor.transpose(pt_[:], vec_ap, idn[:P, :P])
                row = tp.tile([1, P], f32, tag="prow", name="prow")
                nc.vector.tensor_copy(row[:], pt_[:])
                sc_ = tp.tile([1, 1], f32, tag="pscl", name="pscl")
                nc.vector.tensor_reduce(sc_[:], row[:], axis=AX.X, op=op)
                return sc_

            def bcast_col(scalar_ap, name):
                pb_ = pp_tp.tile([P, 1], f32, tag="tp", name=f"pb_{name}")
                nc.tensor.matmul(pb_[:], onr[:1, :P], scalar_ap, start=True,
                                 stop=True)
                return pb_

            for rnd in range(2):
                # probes t_i = lo + (i+1)*(hi-lo)/63
                stp = tp.tile([P, 1], f32, tag="stp", name="stp")
                nc.vector.tensor_sub(stp[:], hi[:], lo[:])
                nc.vector.tensor_scalar(stp[:], stp[:], 1.0 / 63.0, None,
                                        OP.mult)
                tcol = tp.tile([P, 1], f32, tag="tcol", name="tcol")
                nc.vector.scalar_tensor_tensor(tcol[:], ioc1[:], stp[:],
                                               lo[:], OP.mult, OP.add)
                nbt = tp.tile([P, 1], f32, tag="nbt", name="nbt")
                nc.vector.tensor_scalar(nbt[:], tcol[:], -1.0, None, OP.mult)
                ssg = tp.tile([P, 1], f32, tag="ssg", name="ssg")
                nc.scalar.activation(sgn[:], R[:], AF.Sign, bias=nbt[:],
                                     accum_out=ssg[:])
                cnt = tp.tile([P, 1], f32, tag="cnt", name="cnt")
                nc.vector.tensor_scalar(cnt[:], ssg[:], 0.5, FLAT / 2.0,
                                        OP.mult, OP.add)
                # candidate columns: [lo-cand, -hi-cand, t*-cand]
                cand = tp.tile([P, 4], f32, tag="cand", name="cand")
                mlo = tp.tile([P, 1], f32, tag="mlo", name="mlo")
                nc.vector.tensor_scalar(mlo[:], cnt[:], 62.75, None, OP.is_ge)
                nc.vector.tensor_mul(cand[:, 0:1], tcol[:], mlo[:])
                mhi = tp.tile([P, 1], f32, tag="mhi", name="mhi")
                nc.vector.tensor_scalar(mhi[:], cnt[:], 62.25, None, OP.is_le)
                hc = tp.tile([P, 1], f32, tag="hc", name="hc")
                nc.vector.tensor_mul(hc[:], tcol[:], mhi[:])
                hc2 = tp.tile([P, 1], f32, tag="hc2", name="hc2")
                nc.vector.tensor_scalar(hc2[:], mhi[:], BIG, -BIG, OP.mult,
                                        OP.add)
                # cand1 = -(t*mhi + BIG*(1-mhi)) = hc2 - hc   (hc2 above is
                # mhi*BIG - BIG = -(BIG*(1-mhi)))
                nc.vector.tensor_sub(cand[:, 1:2], hc2[:], hc[:])
                c62 = tp.tile([P, 1], f32, tag="c62", name="c62")
                nc.vector.tensor_scalar(c62[:], cnt[:], -62.0, None, OP.add)
                sq = tp.tile([P, 1], f32, tag="sq", name="sq")
                nc.vector.tensor_mul(sq[:], c62[:], c62[:])
                meq = tp.tile([P, 1], f32, tag="meq", name="meq")
                nc.vector.tensor_scalar(meq[:], sq[:], 0.07, None, OP.is_le)
                nc.vector.tensor_mul(cand[:, 2:3], tcol[:], meq[:])
                nc.vector.memset(cand[:, 3:4], 0.0)
                # one transpose + one max-reduce handles all three updates
                pcd = pp_tp.tile([4, P], f32, tag="tp", name=f"pcd{rnd}")
                nc.tensor.transpose(pcd[:], cand[:], idn[:P, :P])
                cdr = tp.tile([4, P], f32, tag="cdr", name="cdr")
                nc.vector.tensor_copy(cdr[:], pcd[:])
                mx3 = tp.tile([4, 1], f32, tag="mx3", name="mx3")
                nc.vector.tensor_reduce(mx3[:], cdr[:], axis=AX.X, op=OP.max)
                mx3r = tp.tile([1, 4], f32, tag="mx3r", name="mx3r")
                pmx3 = pp_tp.tile([1, 4], f32, tag="tp", name=f"pmx3{rnd}")
                nc.tensor.transpose(pmx3[:], mx3[:], idn[:4, :4])
                nc.vector.tensor_copy(mx3r[:], pmx3[:])
                pbc3 = pp_tp.tile([P, 4], f32, tag="tp", name=f"pbc3{rnd}")
                nc.tensor.matmul(pbc3[:], onr[:1, :P], mx3r[:1, :],
                                 start=True, stop=True)
                nc.vector.tensor_max(lo[:], lo[:], pbc3[:, 0:1])
                nhi = tp.tile([P, 1], f32, tag="nhi", name="nhi")
                nc.vector.tensor_scalar(nhi[:], pbc3[:, 1:2], -1.0, None,
                                        OP.mult)
                nc.vector.tensor_tensor(hi[:], hi[:], nhi[:], OP.min)
                nc.vector.tensor_max(tstar[:], tstar[:], pbc3[:, 2:3])
            # (3) mask / compaction ranks / one-hots (verified scheme)
            mask = wp.tile([P, P], f32, tag="mask")
            nc.vector.tensor_scalar(mask[:], PA[:], tstar[:], None, OP.is_gt)
            pmT = pp_sel.tile([P, P], f32, tag="ps", name="pmT")
            nc.tensor.transpose(pmT[:], mask[:], idn[:P, :P])
            mT = wp.tile([P, P], f32, tag="mT")
            nc.vector.tensor_copy(mT[:], pmT[:])
            prc = pp_sel.tile([P, P], f32, tag="ps", name="prc")
            nc.tensor.matmul(prc[:], mT[:], lt[:], start=True, stop=True)
            rcm = wp.tile([P, P], f32, tag="rcm")
            nc.vector.tensor_copy(rcm[:], prc[:])
            pro = pp_tp.tile([1, P], f32, tag="tp", name="pro")
            nc.tensor.matmul(pro[:], rcm[:, P - 1:P], slt[:], start=True,
                             stop=True)
            ror = wp.tile([1, P], f32, tag="ror")
            nc.vector.tensor_copy(ror[:], pro[:])
            proc = pp_tp.tile([P, 1], f32, tag="tp", name="proc")
            nc.tensor.transpose(proc[:], ror[:1, :], idn[:1, :1])
            roc = wp.tile([P, 1], f32, tag="roc")
            nc.vector.tensor_copy(roc[:], proc[:])
            re_ = wp.tile([P, 1], f32, tag="re")
            nc.vector.tensor_add(re_[:], roc[:], rcm[:, P - 1:P])
            g1 = tp.tile([P, P], f32, tag="selu_m", name="g1")
            nc.vector.tensor_scalar(g1[:], iof[:], roc[:], None, OP.is_ge)
            g2 = tp.tile([P, P], f32, tag="selu_r", name="g2")
            nc.vector.tensor_scalar(g2[:], iof[:], re_[:], None, OP.is_lt)
            bb = wp.tile([P, P], f32, tag="bb")
            nc.vector.tensor_mul(bb[:], g1[:], g2[:])
            t1 = tp.tile([P, P], f32, tag="selu_e", name="t1")
            nc.vector.tensor_scalar(t1[:], rcm[:], roc[:], None, OP.add)
            t2 = tp.tile([P, P], f32, tag="selu_e2", name="t2")
            nc.vector.tensor_mul(t2[:], t1[:], mask[:])
            t3 = tp.tile([P, P], f32, tag="selu_m", name="t3")
            nc.vector.tensor_scalar(t3[:], t2[:], -1.0, None, OP.add)
            idx = wp.tile([64, P], i16, tag="idx")
            nc.vector.memset(idx[:], -1)
            nc.vector.tensor_copy(idx[:P, :], t3[:])
            scx = wp.tile([64, 64], bf16, tag="scx")
            nc.gpsimd.local_scatter(scx[:], iofb[:], idx[:], channels=64,
                                    num_elems=64, num_idxs=P)
            pas = pp_tp.tile([1, P], f32, tag="tp", name="pas")
            nc.tensor.matmul(pas[:], ocb[:], scx[:P, :P], start=True,
                             stop=True)
            asr = wp.tile([1, P], f32, tag="asr")
            nc.vector.tensor_copy(asr[:], pas[:])
            pab = pp_sel.tile([P, P], f32, tag="ps", name="pab")
            nc.tensor.matmul(pab[:], onr[:1, :P], asr[:1, :], start=True,
                             stop=True)
            ba = wp.tile([P, P], f32, tag="ba")
            nc.vector.tensor_scalar(ba[:], pab[:], ioc[:], None, OP.is_equal)
            pz = pp_sel.tile([P, P], f32, tag="ps", name="pz")
            nc.tensor.matmul(pz[:], bb[:], PA[:], start=True, stop=True)
            pbat = pp_sel.tile([P, P], f32, tag="ps", name="pbat")
            nc.tensor.transpose(pbat[:], ba[:], idn[:P, :P])
            bat = wp.tile([P, P], f32, tag="bat")
            nc.vector.tensor_copy(bat[:], pbat[:])
            pmm = tp.tile([P, P], f32, tag="selu_r", name="pmm")
            nc.vector.tensor_mul(pmm[:], pz[:], bat[:])
            pcol = wp.tile([P, 1], f32, tag="pcol")
            nc.vector.tensor_reduce(pcol[:], pmm[:], axis=AX.X, op=OP.add)

            # ---------------- stage 3 (tenor) ----------------
            h1t = []
            for mt in range(MT):
                pg = pp_l1.tile([128, P], f32, tag="l2", name="pg")
                nc.tensor.matmul(pg[:], tohb[:, mt * 128:(mt + 1) * 128],
                                 bb[:], start=True, stop=False)
                nc.tensor.matmul(pg[:], toha[:, mt * 128:(mt + 1) * 128],
                                 ba[:], start=False, stop=True)
                h1t.append(selu_chain(pg[:], sht[mt][:], 128, P, f"h1t{mt}"))

            ht1 = dp.tile([HS, P], f32, tag="ht1")
            for mt in range(MT):
                nc.gpsimd.dma_start(ht1[mt * 128:(mt + 1) * 128, :],
                                    h1t[mt][:])
            ght = dp.tile([H, P], f32, tag="ght")
            nc.gpsimd.collective_compute(
                "AllGather", OP.bypass, replica_groups=RG,
                ins=[ht1[:].opt()], outs=[ght[:].opt()])
            H1tT = wp.tile([128, KT2 * P], f32, tag="HT", bufs=2, name="H1tT")
            nc.sync.dma_start(
                H1tT[:].rearrange("p (kt w) -> p kt w", w=P),
                ght[:].rearrange("(kt p) w -> p kt w", p=128))

            h2t = []
            for mt in range(MT):
                pyt = pp_l1.tile([128, P], f32, tag="l2", name="pyt")
                for kt in range(KT2):
                    nc.tensor.matmul(
                        pyt[:],
                        w2s["t"][:, kt * HS + mt * 128:kt * HS + (mt + 1) * 128],
                        H1tT[:, kt * P:(kt + 1) * P],
                        start=(kt == 0), stop=False)
                nc.tensor.matmul(pyt[:], b2s["t"][:1, mt * 128:(mt + 1) * 128],
                                 onr[:1, :P], start=False, stop=True)
                h2t.append(selu_chain(pyt[:], None, 128, P, f"h2t{mt}"))

            ht2 = dp.tile([HS, P], f32, tag="ht2")
            for mt in range(MT):
                nc.gpsimd.dma_start(ht2[mt * 128:(mt + 1) * 128, :],
                                    h2t[mt][:])
            ght2 = dp.tile([H, P], f32, tag="ght2")
            nc.gpsimd.collective_compute(
                "AllGather", OP.bypass, replica_groups=RG,
                ins=[ht2[:].opt()], outs=[ght2[:].opt()])
            H2tT = wp.tile([128, KT2 * P], f32, tag="HT", bufs=2, name="H2tT")
            nc.sync.dma_start(
                H2tT[:].rearrange("p (kt w) -> p kt w", w=P),
                ght2[:].rearrange("(kt p) w -> p kt w", p=128))

            plg_t = pp_sel.tile([P, P], f32, tag="ps", name="plg_t")
            for kt in range(KT2):
                nc.tensor.matmul(plg_t[:], H2tT[:, kt * P:(kt + 1) * P],
                                 w3s["t"][:, kt * P:(kt + 1) * P],
                                 start=(kt == 0), stop=False)
            nc.tensor.matmul(plg_t[:], onr[:1, :P], b3s["t"][:1, :],
                             start=False, stop=True)
            S3 = wp.tile([P, P], f32, tag="S3")
            nc.vector.tensor_copy(S3[:], plg_t[:])
            nm3 = wp.tile([P, 1], f32, tag="nm3")
            nc.vector.tensor_reduce(nm3[:], S3[:], axis=AX.X, op=OP.max,
                                    negate=True)
            E3 = wp.tile([P, P], f32, tag="E3")
            ssum3 = wp.tile([P, 1], f32, tag="ssum3")
            nc.scalar.activation(E3[:], S3[:], AF.Exp, bias=nm3[:],
                                 accum_out=ssum3[:])
            rec3 = wp.tile([P, 1], f32, tag="rec3")
            nc.vector.reciprocal(rec3[:], ssum3[:])
            vv = wp.tile([P, 1], f32, tag="vv")
            nc.vector.tensor_mul(vv[:], rec3[:], pcol[:])
            PT = wp.tile([P, P], f32, tag="PT")
            nc.vector.tensor_scalar(PT[:], E3[:], vv[:], None, OP.mult)
            nc.scalar.dma_start(pt_out[:], PT[:])

    nc.compile()
    return nc


_NC_CACHE = None


def _get_nc():
    global _NC_CACHE
    if _NC_CACHE is None:
        _NC_CACHE = _build()
    return _NC_CACHE


def _prep_inputs(inputs):
    lam = np.float32(LAM)
    x = np.asarray(inputs["inputs_bass"], np.float32)

    def w1img(w):
        # [D, 256] -> [128, KT1*256]: img[p, t*256+m] = w[t*128+p, m]
        return np.ascontiguousarray(
            w.reshape(KT1, 128, HS).transpose(1, 0, 2).reshape(128, KT1 * HS))

    def w2img(w):
        return np.ascontiguousarray(
            w.reshape(KT2, 128, HS).transpose(1, 0, 2).reshape(128, KT2 * HS))

    def w3img(w):
        # [2048, P] -> [128, KT2*P]
        return np.ascontiguousarray(
            w.reshape(KT2, 128, P).transpose(1, 0, 2).reshape(128, KT2 * P))

    def mtimg(w):
        # [256, P] -> [128, MT*P]
        return np.ascontiguousarray(
            w.reshape(MT, 128, P).transpose(1, 0, 2).reshape(128, MT * P))

    base = {
        "ident": np.eye(128, dtype=np.float32),
        "LTc": (np.arange(P)[:, None] <= np.arange(P)[None, :]).astype(np.float32),
        "SLTc": (np.arange(P)[:, None] < np.arange(P)[None, :]).astype(np.float32),
        "iotaF": np.broadcast_to(np.arange(P, dtype=np.float32), (P, P)).copy(),
        "iotaC": np.arange(P, dtype=np.float32)[:, None].copy(),
        "iotaC1": (np.arange(P, dtype=np.float32)[:, None] + 1.0).copy(),
        "onesR": np.ones((1, HS), np.float32),
        "onesCbf": np.ones((P, 1), ml_dtypes.bfloat16),
        "iotaFbf": np.broadcast_to(
            np.arange(P, dtype=ml_dtypes.bfloat16), (64, P)).copy(),
        "xT": np.ascontiguousarray(x.reshape(KT1, 128).T),
        "one1": np.ones((1, 1), np.float32),
    }
    W = {k: np.asarray(v, np.float32) for k, v in inputs.items()}
    in_maps = []
    for c in range(NCORES):
        cols = slice(HS * c, HS * (c + 1))
        m = dict(base)
        for s in "bat":
            m[f"{s}w1i"] = w1img(lam * W[f"{s}w1"][:D, cols])
            m[f"{s}w2i"] = w2img(lam * W[f"{s}w2"][:, cols])
            m[f"{s}w3i"] = w3img(W[f"{s}w3"])
            m[f"{s}b1r"] = (lam * W[f"{s}b1"][cols])[None, :].copy()
            m[f"{s}b2r"] = (lam * W[f"{s}b2"][cols])[None, :].copy()
            m[f"{s}b3r"] = W[f"{s}b3"][None, :].copy()
        m["aohT"] = mtimg(np.ascontiguousarray(
            (lam * W["aw1"][D:D + P, cols]).T))
        m["tohb"] = np.ascontiguousarray(lam * W["tw1"][D:D + P, cols])
        m["toha"] = np.ascontiguousarray(lam * W["tw1"][D + P:D + 2 * P, cols])
        in_maps.append(m)
    return in_maps


def _postprocess(pa, pt):
    flat = pa.reshape(-1)
    order = np.argsort(-flat, kind="stable")[:P]
    sel = np.sort(order)                  # device rank order = flat position
    j_sel = sel // P
    a_sel = sel % P
    flat3 = pt.reshape(-1)
    idx3 = np.argsort(-flat3, kind="stable")[:P]
    row = idx3 // P
    out = np.stack([
        flat3[idx3],
        j_sel[row].astype(np.float32),
        a_sel[row].astype(np.float32),
        (idx3 % P).astype(np.float32),
    ], axis=1)
    return out


def run(inputs, trace=False):
    nc = _get_nc()
    in_maps = _prep_inputs(inputs)
    res = bass_utils.run_bass_kernel_spmd(
        nc, in_maps, core_ids=list(range(NCORES)), trace=trace)
    r0 = res.results[0]
    out = _postprocess(r0["pa_out"], r0["pt_out"])
    return out, res.exec_time_ns


def kernel(**inputs) -> np.ndarray:
    out, _ = run(inputs, trace=False)
    return out
